# revision 3
# baseline (speedup 1.0000x reference)
"""Trainium2 Bass kernel for EquivariantTPConv (gnn_message_passing), v4.

Computation per edge e:
  sh  = SH_l012(edge_vec[e])                                  # [9]
  w   = (silu(edge_scalars[e] @ W1 + b1) @ W2 + b2)           # [3*64*16]
  x   = h_src[src_idx[e]]                                     # [64]
  feat[l,v] = sum_u x[u] * w[l,u,v] / 8                       # [3,16]
  msg = concat_l (feat[:,l,:,None] * sh_l[None,:])            # [144]
  out[d] = mean over {e: dst_idx[e]==d} msg[e]                # [n_dst,144]

v4 strategy (vs the v1 elementwise pipeline):
  - mm2 is emitted TRANSPOSED: 24 matmuls per tile with lhsT = W2p column
    block [128h, 128] and rhs = h2 [128h, 128e], giving wT[(l,v,u)-slice,
    e] in PSUM.  Each 128-partition block holds 2 (l,v) pairs x 64 u.
  - PSUM->SBUF bf16 cast of wT is split between ScalarE and GpSimd.
  - VectorE multiplies by xrep (x^T duplicated on 128 partitions, host-
    prepared, pre-scaled by 1/cnt[dst]) in 2x bf16 mode -> prodT.
  - The u-reduction is 24 tiny PE matmuls: lhsT = prodT block [128, 128e],
    rhs = E2 [128, 2] block-ones -> feat[T, 2] each, written into a shared
    feat PSUM region.  The b2 bias lands there too via one extra matmul
    with lhsT = xrep and rhs = b2 rows (b2 pre-divided into the two
    64-row halves).
  - msg outer products with SH on GpSimd read feat straight from PSUM;
    scatter uses the one-hot matmul; 1/cnt is pre-folded into xrep so the
    eviction is a plain copy on VectorE.
  - Host unshard: segment-sum of slot rows across tiles/cores (unchanged).
"""

import sys

for _p in ("/opt/trn_rl_repo", "/root/.axon_site/_ro/trn_rl_repo"):
    if _p not in sys.path:
        sys.path.append(_p)

import numpy as np

MUL_SRC = 64
MUL_DST = 16
N_PATHS = 3
SQ3 = 3.0 ** 0.5
SQ5 = 5.0 ** 0.5
SQ15 = 15.0 ** 0.5

N_CORES = 8
E_TOT = 50000
N_SRC = 10000
N_DST = 10000
ESD = 32
HID = 128
WCOLS = N_PATHS * MUL_DST * MUL_SRC  # 3072 (perm layout (l,v,u))
NLV = N_PATHS * MUL_DST  # 48

T = 128  # edges per tile
BLK = 512  # edges per full MM1 block (last block is a 128-edge tail)
EC = E_TOT // N_CORES  # 6250 edges per core
NT = (EC + T - 1) // T  # 49 tiles
EP = NT * T  # 6272 padded edges per core
NB = (EP + BLK - 1) // BLK  # 13 blocks, last one covers a single tile

NSB = WCOLS // T  # 24 (l,v,u) column blocks of 128 per tile
NSH = NSB // 2  # 12 blocks per half
HW = NSH * T  # 1536 cols per half

# per half: blocks 0..7 (1024 cols) go ScalarE-cast + VectorE-mult via pwS;
# blocks 8..11 (512 cols) go GpSimd fused cast*mult straight from pwP PSUM.
NS_S = 8  # pwS blocks per half
CS = NS_S * T  # 1024
NS_P = NSH - NS_S  # 4
CP = NS_P * T  # 512

_PROG = None  # cached compiled program


def _build_program():
    from contextlib import ExitStack

    import concourse.tile as tile
    from concourse import bacc, mybir

    f32 = mybir.dt.float32
    bf16 = mybir.dt.bfloat16
    AF = mybir.ActivationFunctionType
    OP = mybir.AluOpType
    AX = mybir.AxisListType

    nc = bacc.Bacc(
        "TRN2",
        target_bir_lowering=False,
        debug=False,
        enable_asserts=False,
        num_devices=N_CORES,
    )

    # DRAM inputs; all big per-core tensors are partition-major on the host.
    esT_d = nc.dram_tensor("esT", [ESD, EP], bf16, kind="ExternalInput")
    xrep_d = nc.dram_tensor("xrep", [HID, EP], bf16, kind="ExternalInput")
    ev_d = nc.dram_tensor("ev", [T, NT, 3], f32, kind="ExternalInput")
    W1_d = nc.dram_tensor("W1", [ESD, HID], bf16, kind="ExternalInput")
    b1_d = nc.dram_tensor("b1", [HID, 1], f32, kind="ExternalInput")
    W2_d = nc.dram_tensor("W2p", [HID, WCOLS], bf16, kind="ExternalInput")
    b2_d = nc.dram_tensor("b2r", [HID, NLV], bf16, kind="ExternalInput")
    e2_d = nc.dram_tensor("e2", [HID, 2], bf16, kind="ExternalInput")
    shc_d = nc.dram_tensor("shc", [T, 8], f32, kind="ExternalInput")
    out_d = nc.dram_tensor("outp", [T, NT, 144], bf16, kind="ExternalOutput")

    with ExitStack() as ctx:
        tc = ctx.enter_context(tile.TileContext(nc))

        const = ctx.enter_context(tc.tile_pool(name="const", bufs=1))
        shp = ctx.enter_context(tc.tile_pool(name="shp", bufs=1))
        h2pool = ctx.enter_context(tc.tile_pool(name="h2pool", bufs=2))
        wbp = ctx.enter_context(tc.tile_pool(name="wbp", bufs=3))
        prp = ctx.enter_context(tc.tile_pool(name="prp", bufs=4))
        msgp = ctx.enter_context(tc.tile_pool(name="msgp", bufs=6))
        ph1 = ctx.enter_context(tc.tile_pool(name="ph1", bufs=1, space="PSUM"))
        pwt = ctx.enter_context(tc.tile_pool(name="pwt", bufs=2, space="PSUM"))
        pfs = ctx.enter_context(tc.tile_pool(name="pfs", bufs=1, space="PSUM"))

        # ---- resident loads; SP carries the edge streams, GpSimd's queue
        # carries the weights, ordered by first use ----
        W1s = const.tile([ESD, HID], bf16)
        nc.gpsimd.dma_start(W1s[:], W1_d.ap())
        b1s = const.tile([HID, 1], f32)
        nc.gpsimd.dma_start(b1s[:], b1_d.ap())
        W2s = const.tile([HID, WCOLS], bf16)
        nc.gpsimd.dma_start(W2s[:, 0:1024], W2_d.ap()[:, 0:1024])
        nc.gpsimd.dma_start(W2s[:, 1024:], W2_d.ap()[:, 1024:])
        b2s = const.tile([HID, NLV], bf16)
        nc.gpsimd.dma_start(b2s[:], b2_d.ap())
        e2s = const.tile([HID, 2], bf16)
        nc.gpsimd.dma_start(e2s[:], e2_d.ap())
        shcs = const.tile([T, 8], f32)
        nc.gpsimd.dma_start(shcs[:], shc_d.ap())

        es_all = const.tile([ESD, EP], bf16)
        nc.sync.dma_start(es_all[:, 0:BLK], esT_d.ap()[:, 0:BLK])
        xrep_all = const.tile([HID, EP], bf16)
        nc.sync.dma_start(xrep_all[:, 0 : 4 * T], xrep_d.ap()[:, 0 : 4 * T])
        ev_all = const.tile([T, NT, 3], f32)
        nc.sync.dma_start(ev_all[:], ev_d.ap())

        nc.sync.dma_start(es_all[:, BLK:], esT_d.ap()[:, BLK:])
        nc.sync.dma_start(xrep_all[:, 4 * T :], xrep_d.ap()[:, 4 * T :])
        negone = const.tile([T, 1], f32)
        nc.vector.memset(negone[:], -1.0)


        # ---- SH prologue: all edges at once, [128, NT, k] layouts ----
        sq_all = shp.tile([T, NT, 3], f32)
        nc.vector.tensor_tensor(sq_all[:], ev_all[:], ev_all[:], op=OP.mult)
        r2_all = shp.tile([T, NT], f32)
        nc.vector.tensor_reduce(r2_all[:], sq_all[:], axis=AX.X, op=OP.add)
        rn_all = shp.tile([T, NT], f32)
        nc.scalar.activation(rn_all[:], r2_all[:], AF.Sqrt)

        def bc(ap_, shape):
            return ap_.to_broadcast(shape)

        sh_all = shp.tile([T, NT, 9], f32)

        def emit_sh_part2():
            inv_all = shp.tile([T, NT], f32)
            nc.vector.reciprocal(inv_all[:], rn_all[:])
            inv2_all = shp.tile([T, NT], f32)
            nc.vector.tensor_tensor(inv2_all[:], inv_all[:], inv_all[:], op=OP.mult)
            i1 = inv_all[:].rearrange("p (t o) -> p t o", o=1)
            i2 = inv2_all[:].rearrange("p (t o) -> p t o", o=1)
            nc.vector.tensor_tensor(
                sh_all[:, :, 1:4], ev_all[:], bc(i1, [T, NT, 3]), op=OP.mult
            )
            pq_all = shp.tile([T, NT, 2], f32)
            nc.vector.tensor_tensor(
                pq_all[:], ev_all[:, :, 0:2], ev_all[:, :, 1:3], op=OP.mult
            )
            nc.vector.tensor_tensor(
                sh_all[:, :, 4:6], pq_all[:], bc(i2, [T, NT, 2]), op=OP.mult
            )
            t6_all = shp.tile([T, NT], f32)
            nc.vector.tensor_tensor(
                t6_all[:].rearrange("p (t o) -> p t o", o=1),
                sq_all[:, :, 2:3],
                i2,
                op=OP.mult,
            )
            nc.scalar.activation(
                sh_all[:, :, 6], t6_all[:], AF.Identity, bias=negone[:, 0:1], scale=3.0
            )
            xz_all = shp.tile([T, NT, 1], f32)
            nc.vector.tensor_tensor(
                xz_all[:], ev_all[:, :, 0:1], ev_all[:, :, 2:3], op=OP.mult
            )
            nc.vector.tensor_tensor(sh_all[:, :, 7:8], xz_all[:], i2, op=OP.mult)
            d2_all = shp.tile([T, NT, 1], f32)
            nc.vector.tensor_tensor(
                d2_all[:], sq_all[:, :, 0:1], sq_all[:, :, 1:2], op=OP.subtract
            )
            nc.vector.tensor_tensor(sh_all[:, :, 8:9], d2_all[:], i2, op=OP.mult)
            shc3 = shcs[:].rearrange("p (o c) -> p o c", o=1)
            nc.vector.tensor_tensor(
                sh_all[:, :, 1:9], sh_all[:, :, 1:9], bc(shc3, [T, NT, 8]), op=OP.mult
            )

        # ---- main pipeline ----
        # stage A(t): mm2 halves -> pwt PSUM; casts (S/P); mult (V) -> prodT
        # stage B(t-1): bias-mm + 24 reduce-mms -> feat region of fp
        #               gpsimd msg outers from feat PSUM; l0 copy
        # stage C(t-2): scatter-mm -> ps region of fp; V evict -> ob_all
        prodT_by_t = {}
        feat_by_t = {}
        msg_by_t = {}

        # single PSUM bank shared by feat [0:48] and the scatter out [64:208]
        fp = pfs.tile([T, 512], f32, tag="fp", name="fp")

        def emit_mm1(b):
            nbt = min(4, NT - b * 4)
            bw = nbt * T
            h1 = ph1.tile([HID, BLK], f32, tag="h1", name=f"h1_{b}")
            nc.tensor.matmul(
                h1[:, 0:bw],
                W1s[:],
                es_all[:, b * BLK : b * BLK + bw],
                start=True,
                stop=True,
            )
            h2 = h2pool.tile([HID, BLK], bf16, tag="h2", name=f"h2_{b}")
            nc.scalar.activation(h2[:, 0:bw], h1[:, 0:bw], AF.Silu, bias=b1s[:, 0:1])
            emit_mm1.h2_by_b[b] = h2

        emit_mm1.h2_by_b = {}

        def emit_tile_head(t):
            b, q = divmod(t, 4)
            h2 = emit_mm1.h2_by_b[b]
            if q == 3 and b - 1 in emit_mm1.h2_by_b:
                del emit_mm1.h2_by_b[b - 1]
            prodT = prp.tile([HID, WCOLS], bf16, tag="prodT", name=f"prodT{t}")
            xe = xrep_all[:, t * T : (t + 1) * T]
            xb1 = xe.rearrange("p (o e) -> p o e", o=1)
            rhs = h2[:, q * T : (q + 1) * T]
            # all pwS matmuls first so the S casts never wait behind pwP work
            pwS_h, pwP_h = [], []
            for hh in range(2):
                pwS = pwt.tile([HID, CS], f32, tag="pwS", name=f"pwS{t}_{hh}", bufs=2)
                pwS_h.append(pwS)
                for j in range(NS_S):
                    s = hh * NSH + j
                    nc.tensor.matmul(
                        pwS[:, j * T : (j + 1) * T],
                        W2s[:, s * T : (s + 1) * T],
                        rhs,
                        start=True,
                        stop=True,
                    )
            for hh in range(2):
                pwP = pwt.tile([HID, CP], f32, tag="pwP", name=f"pwP{t}_{hh}", bufs=2)
                pwP_h.append(pwP)
                for j in range(NS_P):
                    s = hh * NSH + NS_S + j
                    nc.tensor.matmul(
                        pwP[:, j * T : (j + 1) * T],
                        W2s[:, s * T : (s + 1) * T],
                        rhs,
                        start=True,
                        stop=True,
                    )
            for hh in range(2):
                wb = wbp.tile([HID, CS], bf16, tag="wb", name=f"wb{t}_{hh}")
                nc.scalar.activation(wb[:], pwS_h[hh][:], AF.Copy)
                # GpSimd fused cast*mult straight from PSUM for blocks 8..11
                pp = prodT[:, hh * HW + CS : (hh + 1) * HW].rearrange(
                    "p (s e) -> p s e", e=T
                )
                nc.vector.tensor_tensor(
                    pp,
                    pwP_h[hh][:].rearrange("p (s e) -> p s e", e=T),
                    xb1.to_broadcast([HID, NS_P, T]),
                    op=OP.mult,
                )
                # VectorE 2x bf16 multiply for the ScalarE-cast blocks
                po = prodT[:, hh * HW : hh * HW + CS - 256].rearrange(
                    "p (s e) -> p s e", e=T
                )
                nc.vector.tensor_tensor(
                    po,
                    wb[:, 0 : CS - 256].rearrange("p (s e) -> p s e", e=T),
                    xb1.to_broadcast([HID, NS_S - 2, T]),
                    op=OP.mult,
                )
                nc.gpsimd.tensor_tensor(
                    prodT[:, hh * HW + CS - 256 : hh * HW + CS].rearrange(
                        "p (s e) -> p s e", e=T
                    ),
                    wb[:, CS - 256 : CS].rearrange("p (s e) -> p s e", e=T),
                    xb1.to_broadcast([HID, 2, T]),
                    op=OP.mult,
                )
            prodT_by_t[t] = prodT

        def emit_tile_reduce(t):
            prodT = prodT_by_t.pop(t)
            feat = fp[:, 256 * (t % 2) : 256 * (t % 2) + NLV]
            # bias: lhsT = xrep (full 128 partitions), rhs = b2 rows (/1 in
            # rows 0:64, zeros in 64:128)
            nc.tensor.matmul(
                feat, xrep_all[:, t * T : (t + 1) * T], b2s[:], start=True, stop=False
            )
            off = 256 * (t % 2)
            for s in range(NSB):
                nc.tensor.matmul(
                    fp[:, off + 2 * s : off + 2 * s + 2],
                    prodT[:, s * T : (s + 1) * T],
                    e2s[:],
                    start=False,
                    stop=True,
                )
            feat_by_t[t] = feat

        def emit_tile_msg(t):
            feat = feat_by_t.pop(t)
            msg = msgp.tile([T, 144], bf16, tag="msg", name=f"msg{t}")
            featc = msgp.tile([T, NLV], bf16, tag="featc", name=f"featc{t}")
            nc.vector.tensor_copy(featc[:], feat)
            nc.gpsimd.tensor_copy(msg[:, 0:16], featc[:, 0:16])
            nc.gpsimd.tensor_tensor(
                msg[:, 16:64].rearrange("p (v m) -> p v m", m=3),
                featc[:, 16:32]
                .rearrange("p (v o) -> p v o", o=1)
                .to_broadcast([T, 16, 3]),
                sh_all[:, t, 1:4]
                .rearrange("p (o m) -> p o m", o=1)
                .to_broadcast([T, 16, 3]),
                op=OP.mult,
            )
            nc.gpsimd.tensor_tensor(
                msg[:, 64:144].rearrange("p (v m) -> p v m", m=5),
                featc[:, 32:48]
                .rearrange("p (v o) -> p v o", o=1)
                .to_broadcast([T, 16, 5]),
                sh_all[:, t, 4:9]
                .rearrange("p (o m) -> p o m", o=1)
                .to_broadcast([T, 16, 5]),
                op=OP.mult,
            )
            msg_by_t[t] = msg

        def emit_tile_scatter(t):
            msg = msg_by_t.pop(t)
            nc.sync.dma_start(out_d.ap()[:, t, :], msg[:])

        emit_mm1(0)
        for t in range(NT):
            # tail work for old tiles first: all inputs are >=1 iteration old,
            # so no engine stalls at the head of its in-order stream
            if t >= 2:
                emit_tile_reduce(t - 2)
                emit_tile_msg(t - 2)
            emit_tile_head(t)
            if (t + 1) % 4 == 0 and t + 1 < NT:
                emit_mm1((t + 1) // 4)
            if t >= 3:
                emit_tile_scatter(t - 3)
            if t == 1:
                emit_sh_part2()
        for t in range(NT - 2, NT):
            emit_tile_reduce(t)
            emit_tile_msg(t)
        for t in range(NT - 3, NT):
            emit_tile_scatter(t)

        # output: chunked DMAs, small final chunk so the tail drains fast
        pass

    nc.compile()
    return nc


def _get_program():
    global _PROG
    if _PROG is None:
        _PROG = _build_program()
    return _PROG


def _prep_core(c, h_src, edge_vec, edge_scalars, src_idx, dst_idx, inv_cnt):
    """Shard + sort + gather + one-hot build for one core (partition-major)."""
    import ml_dtypes

    bf = ml_dtypes.bfloat16
    lo, hi = c * EC, (c + 1) * EC
    d = dst_idx[lo:hi]
    order = np.argsort(d, kind="stable")
    d_s = d[order]
    s_s = src_idx[lo:hi][order]

    esT = np.zeros((ESD, EP), np.float32)
    esT[:, :EC] = edge_scalars[lo:hi][order].T
    # x rows pre-scaled by 1/cnt[dst] (folds the scatter-mean divide); pads 0
    x = np.zeros((EP, MUL_SRC), np.float32)
    x[:EC] = h_src[s_s] * inv_cnt[d_s][:, None]
    xrep = np.concatenate([x.T, x.T], axis=0)  # [128, EP]
    ev = np.zeros((EP, 3), np.float32)
    ev[:EC] = edge_vec[lo:hi][order]
    ev[EC:, 0] = 1.0

    labels = np.full(EP, N_DST, np.int64)
    labels[:EC] = d_s

    # partition-major device layouts: [p, t, ...] = row t*T + p
    def pmaj(a):
        return np.ascontiguousarray(a.reshape(NT, T, -1).transpose(1, 0, 2))

    return (
        {
            "esT": esT.astype(bf),
            "xrep": np.ascontiguousarray(xrep).astype(bf),
            "ev": pmaj(ev),
        },
        labels,
    )


def kernel(**inputs):
    import ml_dtypes

    from concourse import bass_utils

    bf = ml_dtypes.bfloat16

    h_src = np.asarray(inputs["h_src"], np.float32)
    edge_vec = np.asarray(inputs["edge_vec"], np.float32)
    edge_scalars = np.asarray(inputs["edge_scalars"], np.float32)
    W1 = np.asarray(inputs["W1"], np.float32)
    b1 = np.asarray(inputs["b1"], np.float32)
    W2 = np.asarray(inputs["W2"], np.float32)
    b2 = np.asarray(inputs["b2"], np.float32)
    src_idx = np.asarray(inputs["src_idx"]).astype(np.int64)
    dst_idx = np.asarray(inputs["dst_idx"]).astype(np.int64)
    n_dst = int(inputs["n_dst"])
    assert n_dst == N_DST

    nc = _get_program()

    cnt = np.bincount(dst_idx, minlength=N_DST)
    inv_cnt = (1.0 / np.maximum(cnt, 1)).astype(np.float32)

    # weights in (l,v,u) column order, pre-scaled by 1/sqrt(64)
    scale = 1.0 / np.sqrt(MUL_SRC)
    W2p = (
        W2.reshape(HID, N_PATHS, MUL_SRC, MUL_DST).transpose(0, 1, 3, 2) * scale
    ).reshape(HID, WCOLS)
    # bias path: feat += x @ b2r with b2r = b2 permuted to (u, l, v).  The
    # device does it as matmul(lhsT=xrep, rhs=b2r-padded); xrep rows 64:128
    # duplicate rows 0:64, so the rhs pads rows 64:128 with zeros.
    b2r = np.zeros((HID, NLV), np.float32)
    b2r[0:MUL_SRC] = (
        b2.reshape(N_PATHS, MUL_SRC, MUL_DST).transpose(1, 0, 2) * scale
    ).reshape(MUL_SRC, NLV)

    # E2: rows 0:64 -> col 0, rows 64:128 -> col 1
    e2 = np.zeros((HID, 2), np.float32)
    e2[0:MUL_SRC, 0] = 1.0
    e2[MUL_SRC:, 1] = 1.0

    shc = np.broadcast_to(
        np.array(
            [SQ3, SQ3, SQ3, SQ15, SQ15, 0.5 * SQ5, SQ15, 0.5 * SQ15], np.float32
        ),
        (T, 8),
    ).copy()

    shared = {
        "W1": np.ascontiguousarray(W1).astype(bf),
        "b1": b1.reshape(HID, 1).astype(np.float32),
        "W2p": W2p.astype(bf),
        "b2r": b2r.astype(bf),
        "e2": e2.astype(bf),
        "shc": shc,
    }

    in_maps = []
    labels_all = []
    for c in range(N_CORES):
        m, labels = _prep_core(
            c, h_src, edge_vec, edge_scalars, src_idx, dst_idx, inv_cnt
        )
        m.update(shared)
        in_maps.append(m)
        labels_all.append(labels)

    import time

    t0 = time.perf_counter()
    res = bass_utils.run_bass_kernel_spmd(nc, in_maps, core_ids=list(range(N_CORES)))
    t1 = time.perf_counter()
    kernel.last_device_wall_s = t1 - t0

    # outp is [T, NT, 144] partition-major; row (t, p) lives at [p, t, :]
    rows = np.concatenate(
        [
            res.results[c]["outp"]
            .astype(np.float32)
            .transpose(1, 0, 2)
            .reshape(NT * T, 144)
            for c in range(N_CORES)
        ],
        axis=0,
    )
    labels = np.concatenate(labels_all)

    order = np.argsort(labels, kind="stable")
    lab_s = labels[order]
    rows_s = rows[order]
    starts = np.concatenate(([0], np.flatnonzero(np.diff(lab_s)) + 1))
    sums = np.add.reduceat(rows_s, starts, axis=0)
    out = np.zeros((N_DST + 1, 144), np.float32)
    out[lab_s[starts]] = sums
    return out[:N_DST]


# revision 4
# speedup vs baseline: 1.0637x; 1.0637x over previous
"""Trainium2 Bass kernel for EquivariantTPConv (gnn_message_passing), v4.

Computation per edge e:
  sh  = SH_l012(edge_vec[e])                                  # [9]
  w   = (silu(edge_scalars[e] @ W1 + b1) @ W2 + b2)           # [3*64*16]
  x   = h_src[src_idx[e]]                                     # [64]
  feat[l,v] = sum_u x[u] * w[l,u,v] / 8                       # [3,16]
  msg = concat_l (feat[:,l,:,None] * sh_l[None,:])            # [144]
  out[d] = mean over {e: dst_idx[e]==d} msg[e]                # [n_dst,144]

v4 strategy (vs the v1 elementwise pipeline, 210.6us -> 141.7us):
  - mm2 is emitted TRANSPOSED: 24 matmuls per tile with lhsT = W2p column
    block [128h, 128] and rhs = h2 [128h, 128e], giving wT[(l,v,u)-slice,
    e] in PSUM (pwS 16 blocks + pwV 8 blocks, double-buffered).  Each
    128-partition block holds 2 (l,v) pairs x 64 u.
  - pwS: ScalarE casts PSUM->SBUF bf16 (wb); VectorE (12 blocks) and
    GpSimd (4 blocks) multiply wb by xrep in 2x bf16 mode.  pwV: VectorE
    multiplies straight from PSUM f32 (GpSimd cannot access PSUM).
    xrep = x^T duplicated on 128 partitions, host-prepared, pre-scaled
    by 1/cnt[dst] so the scatter-mean divide is free.
  - The u-reduction is 24 tiny PE matmuls per tile: lhsT = prodT block
    [128, 128e], rhs = E2 [128, 2] block-ones -> feat[T, 2] each,
    accumulated into a shared feat PSUM bank on top of the b2 bias
    (one matmul with lhsT = xrep, rhs = padded b2).
  - feat is evicted to SBUF by VectorE; GpSimd builds msg (l0 copy +
    SH outer products); per-edge msg goes to DRAM in bf16 via DMA.
  - Host does the segment-sum over dst (edges pre-sorted by dst per
    core), replacing the on-device one-hot scatter matmul.
"""

import sys

for _p in ("/opt/trn_rl_repo", "/root/.axon_site/_ro/trn_rl_repo"):
    if _p not in sys.path:
        sys.path.append(_p)

import numpy as np

MUL_SRC = 64
MUL_DST = 16
N_PATHS = 3
SQ3 = 3.0 ** 0.5
SQ5 = 5.0 ** 0.5
SQ15 = 15.0 ** 0.5

N_CORES = 8
E_TOT = 50000
N_SRC = 10000
N_DST = 10000
ESD = 32
HID = 128
WCOLS = N_PATHS * MUL_DST * MUL_SRC  # 3072 (perm layout (l,v,u))
NLV = N_PATHS * MUL_DST  # 48

T = 128  # edges per tile
BLK = 512  # edges per full MM1 block (last block is a 128-edge tail)
EC = E_TOT // N_CORES  # 6250 edges per core
NT = (EC + T - 1) // T  # 49 tiles
EP = NT * T  # 6272 padded edges per core
NB = (EP + BLK - 1) // BLK  # 13 blocks, last one covers a single tile

NSB = WCOLS // T  # 24 (l,v,u) column blocks of 128 per tile
NSH = NSB // 2  # 12 blocks per half
HW = NSH * T  # 1536 cols per half

# per half: blocks 0..7 (1024 cols) go ScalarE-cast + VectorE-mult via pwS;
# blocks 8..11 (512 cols) go GpSimd fused cast*mult straight from pwP PSUM.
NS_S = 8  # pwS blocks per half
CS = NS_S * T  # 1024
NS_P = NSH - NS_S  # 4
CP = NS_P * T  # 512

_PROG = None  # cached compiled program


def _build_program():
    from contextlib import ExitStack

    import concourse.tile as tile
    from concourse import bacc, mybir

    f32 = mybir.dt.float32
    bf16 = mybir.dt.bfloat16
    AF = mybir.ActivationFunctionType
    OP = mybir.AluOpType
    AX = mybir.AxisListType

    nc = bacc.Bacc(
        "TRN2",
        target_bir_lowering=False,
        debug=False,
        enable_asserts=False,
        num_devices=N_CORES,
    )

    # DRAM inputs; all big per-core tensors are partition-major on the host.
    esT_d = nc.dram_tensor("esT", [ESD, EP], bf16, kind="ExternalInput")
    xrep_d = nc.dram_tensor("xrep", [HID, EP], bf16, kind="ExternalInput")
    ev_d = nc.dram_tensor("ev", [T, NT, 3], f32, kind="ExternalInput")
    W1_d = nc.dram_tensor("W1", [ESD, HID], bf16, kind="ExternalInput")
    b1_d = nc.dram_tensor("b1", [HID, 1], f32, kind="ExternalInput")
    W2_d = nc.dram_tensor("W2p", [HID, WCOLS], bf16, kind="ExternalInput")
    b2_d = nc.dram_tensor("b2r", [HID, NLV], bf16, kind="ExternalInput")
    e2_d = nc.dram_tensor("e2", [HID, 2], bf16, kind="ExternalInput")
    shc_d = nc.dram_tensor("shc", [T, 8], f32, kind="ExternalInput")
    out_d = nc.dram_tensor("outp", [T, NT, 144], bf16, kind="ExternalOutput")

    with ExitStack() as ctx:
        tc = ctx.enter_context(tile.TileContext(nc))

        const = ctx.enter_context(tc.tile_pool(name="const", bufs=1))
        shp = ctx.enter_context(tc.tile_pool(name="shp", bufs=1))
        h2pool = ctx.enter_context(tc.tile_pool(name="h2pool", bufs=2))
        wbp = ctx.enter_context(tc.tile_pool(name="wbp", bufs=3))
        prp = ctx.enter_context(tc.tile_pool(name="prp", bufs=4))
        msgp = ctx.enter_context(tc.tile_pool(name="msgp", bufs=6))
        ph1 = ctx.enter_context(tc.tile_pool(name="ph1", bufs=1, space="PSUM"))
        pwt = ctx.enter_context(tc.tile_pool(name="pwt", bufs=2, space="PSUM"))
        pfs = ctx.enter_context(tc.tile_pool(name="pfs", bufs=1, space="PSUM"))

        # ---- resident loads; SP carries the edge streams, GpSimd's queue
        # carries the weights, ordered by first use ----
        W1s = const.tile([ESD, HID], bf16)
        nc.gpsimd.dma_start(W1s[:], W1_d.ap())
        b1s = const.tile([HID, 1], f32)
        nc.gpsimd.dma_start(b1s[:], b1_d.ap())
        W2s = const.tile([HID, WCOLS], bf16)
        nc.gpsimd.dma_start(W2s[:, 0:1024], W2_d.ap()[:, 0:1024])
        nc.gpsimd.dma_start(W2s[:, 1024:], W2_d.ap()[:, 1024:])
        b2s = const.tile([HID, NLV], bf16)
        nc.gpsimd.dma_start(b2s[:], b2_d.ap())
        e2s = const.tile([HID, 2], bf16)
        nc.gpsimd.dma_start(e2s[:], e2_d.ap())
        shcs = const.tile([T, 8], f32)
        nc.gpsimd.dma_start(shcs[:], shc_d.ap())

        es_all = const.tile([ESD, EP], bf16)
        nc.sync.dma_start(es_all[:, 0:BLK], esT_d.ap()[:, 0:BLK])
        xrep_all = const.tile([HID, EP], bf16)
        nc.sync.dma_start(xrep_all[:, 0 : 4 * T], xrep_d.ap()[:, 0 : 4 * T])
        ev_all = const.tile([T, NT, 3], f32)
        nc.sync.dma_start(ev_all[:], ev_d.ap())

        nc.sync.dma_start(es_all[:, BLK:], esT_d.ap()[:, BLK:])
        nc.sync.dma_start(xrep_all[:, 4 * T :], xrep_d.ap()[:, 4 * T :])
        negone = const.tile([T, 1], f32)
        nc.vector.memset(negone[:], -1.0)


        # ---- SH prologue: all edges at once, [128, NT, k] layouts ----
        sq_all = shp.tile([T, NT, 3], f32)
        nc.vector.tensor_tensor(sq_all[:], ev_all[:], ev_all[:], op=OP.mult)
        r2_all = shp.tile([T, NT], f32)
        nc.vector.tensor_reduce(r2_all[:], sq_all[:], axis=AX.X, op=OP.add)
        rn_all = shp.tile([T, NT], f32)
        nc.scalar.activation(rn_all[:], r2_all[:], AF.Sqrt)

        def bc(ap_, shape):
            return ap_.to_broadcast(shape)

        sh_all = shp.tile([T, NT, 9], f32)

        def emit_sh_part2():
            inv_all = shp.tile([T, NT], f32)
            nc.vector.reciprocal(inv_all[:], rn_all[:])
            inv2_all = shp.tile([T, NT], f32)
            nc.vector.tensor_tensor(inv2_all[:], inv_all[:], inv_all[:], op=OP.mult)
            i1 = inv_all[:].rearrange("p (t o) -> p t o", o=1)
            i2 = inv2_all[:].rearrange("p (t o) -> p t o", o=1)
            nc.vector.tensor_tensor(
                sh_all[:, :, 1:4], ev_all[:], bc(i1, [T, NT, 3]), op=OP.mult
            )
            pq_all = shp.tile([T, NT, 2], f32)
            nc.vector.tensor_tensor(
                pq_all[:], ev_all[:, :, 0:2], ev_all[:, :, 1:3], op=OP.mult
            )
            nc.vector.tensor_tensor(
                sh_all[:, :, 4:6], pq_all[:], bc(i2, [T, NT, 2]), op=OP.mult
            )
            t6_all = shp.tile([T, NT], f32)
            nc.vector.tensor_tensor(
                t6_all[:].rearrange("p (t o) -> p t o", o=1),
                sq_all[:, :, 2:3],
                i2,
                op=OP.mult,
            )
            nc.scalar.activation(
                sh_all[:, :, 6], t6_all[:], AF.Identity, bias=negone[:, 0:1], scale=3.0
            )
            xz_all = shp.tile([T, NT, 1], f32)
            nc.vector.tensor_tensor(
                xz_all[:], ev_all[:, :, 0:1], ev_all[:, :, 2:3], op=OP.mult
            )
            nc.vector.tensor_tensor(sh_all[:, :, 7:8], xz_all[:], i2, op=OP.mult)
            d2_all = shp.tile([T, NT, 1], f32)
            nc.vector.tensor_tensor(
                d2_all[:], sq_all[:, :, 0:1], sq_all[:, :, 1:2], op=OP.subtract
            )
            nc.vector.tensor_tensor(sh_all[:, :, 8:9], d2_all[:], i2, op=OP.mult)
            shc3 = shcs[:].rearrange("p (o c) -> p o c", o=1)
            nc.vector.tensor_tensor(
                sh_all[:, :, 1:9], sh_all[:, :, 1:9], bc(shc3, [T, NT, 8]), op=OP.mult
            )

        # ---- main pipeline ----
        # stage A(t): mm2 halves -> pwt PSUM; casts (S/P); mult (V) -> prodT
        # stage B(t-1): bias-mm + 24 reduce-mms -> feat region of fp
        #               gpsimd msg outers from feat PSUM; l0 copy
        # stage C(t-2): scatter-mm -> ps region of fp; V evict -> ob_all
        prodT_by_t = {}
        feat_by_t = {}
        msg_by_t = {}

        # single PSUM bank shared by feat [0:48] and the scatter out [64:208]
        fp = pfs.tile([T, 512], f32, tag="fp", name="fp")

        def emit_mm1(b):
            nbt = min(4, NT - b * 4)
            bw = nbt * T
            h1 = ph1.tile([HID, BLK], f32, tag="h1", name=f"h1_{b}")
            nc.tensor.matmul(
                h1[:, 0:bw],
                W1s[:],
                es_all[:, b * BLK : b * BLK + bw],
                start=True,
                stop=True,
            )
            h2 = h2pool.tile([HID, BLK], bf16, tag="h2", name=f"h2_{b}")
            nc.scalar.activation(h2[:, 0:bw], h1[:, 0:bw], AF.Silu, bias=b1s[:, 0:1])
            emit_mm1.h2_by_b[b] = h2

        emit_mm1.h2_by_b = {}

        def emit_tile_head(t):
            b, q = divmod(t, 4)
            h2 = emit_mm1.h2_by_b[b]
            if q == 3 and b - 1 in emit_mm1.h2_by_b:
                del emit_mm1.h2_by_b[b - 1]
            prodT = prp.tile([HID, WCOLS], bf16, tag="prodT", name=f"prodT{t}")
            xe = xrep_all[:, t * T : (t + 1) * T]
            xb1 = xe.rearrange("p (o e) -> p o e", o=1)
            rhs = h2[:, q * T : (q + 1) * T]
            # all pwS matmuls first so the S casts never wait behind pwP work
            pwS_h, pwP_h = [], []
            for hh in range(2):
                pwS = pwt.tile([HID, CS], f32, tag="pwS", name=f"pwS{t}_{hh}", bufs=2)
                pwS_h.append(pwS)
                for j in range(NS_S):
                    s = hh * NSH + j
                    nc.tensor.matmul(
                        pwS[:, j * T : (j + 1) * T],
                        W2s[:, s * T : (s + 1) * T],
                        rhs,
                        start=True,
                        stop=True,
                    )
            for hh in range(2):
                pwP = pwt.tile([HID, CP], f32, tag="pwP", name=f"pwP{t}_{hh}", bufs=2)
                pwP_h.append(pwP)
                for j in range(NS_P):
                    s = hh * NSH + NS_S + j
                    nc.tensor.matmul(
                        pwP[:, j * T : (j + 1) * T],
                        W2s[:, s * T : (s + 1) * T],
                        rhs,
                        start=True,
                        stop=True,
                    )
            for hh in range(2):
                wb = wbp.tile([HID, CS], bf16, tag="wb", name=f"wb{t}_{hh}")
                nc.scalar.activation(wb[:], pwS_h[hh][:], AF.Copy)
                # GpSimd fused cast*mult straight from PSUM for blocks 8..11
                pp = prodT[:, hh * HW + CS : (hh + 1) * HW].rearrange(
                    "p (s e) -> p s e", e=T
                )
                nc.vector.tensor_tensor(
                    pp,
                    pwP_h[hh][:].rearrange("p (s e) -> p s e", e=T),
                    xb1.to_broadcast([HID, NS_P, T]),
                    op=OP.mult,
                )
                # VectorE 2x bf16 multiply for the ScalarE-cast blocks
                po = prodT[:, hh * HW : hh * HW + CS - 256].rearrange(
                    "p (s e) -> p s e", e=T
                )
                nc.vector.tensor_tensor(
                    po,
                    wb[:, 0 : CS - 256].rearrange("p (s e) -> p s e", e=T),
                    xb1.to_broadcast([HID, NS_S - 2, T]),
                    op=OP.mult,
                )
                nc.gpsimd.tensor_tensor(
                    prodT[:, hh * HW + CS - 256 : hh * HW + CS].rearrange(
                        "p (s e) -> p s e", e=T
                    ),
                    wb[:, CS - 256 : CS].rearrange("p (s e) -> p s e", e=T),
                    xb1.to_broadcast([HID, 2, T]),
                    op=OP.mult,
                )
            prodT_by_t[t] = prodT

        def emit_tile_reduce(t):
            prodT = prodT_by_t.pop(t)
            feat = fp[:, 256 * (t % 2) : 256 * (t % 2) + NLV]
            # bias: lhsT = xrep (full 128 partitions), rhs = b2 rows (/1 in
            # rows 0:64, zeros in 64:128)
            nc.tensor.matmul(
                feat, xrep_all[:, t * T : (t + 1) * T], b2s[:], start=True, stop=False
            )
            off = 256 * (t % 2)
            for s in range(NSB):
                nc.tensor.matmul(
                    fp[:, off + 2 * s : off + 2 * s + 2],
                    prodT[:, s * T : (s + 1) * T],
                    e2s[:],
                    start=False,
                    stop=True,
                )
            feat_by_t[t] = feat

        def emit_tile_msg(t):
            feat = feat_by_t.pop(t)
            msg = msgp.tile([T, 144], bf16, tag="msg", name=f"msg{t}")
            featc = msgp.tile([T, NLV], bf16, tag="featc", name=f"featc{t}")
            nc.vector.tensor_copy(featc[:], feat)
            nc.gpsimd.tensor_copy(msg[:, 0:16], featc[:, 0:16])
            nc.gpsimd.tensor_tensor(
                msg[:, 16:64].rearrange("p (v m) -> p v m", m=3),
                featc[:, 16:32]
                .rearrange("p (v o) -> p v o", o=1)
                .to_broadcast([T, 16, 3]),
                sh_all[:, t, 1:4]
                .rearrange("p (o m) -> p o m", o=1)
                .to_broadcast([T, 16, 3]),
                op=OP.mult,
            )
            nc.gpsimd.tensor_tensor(
                msg[:, 64:144].rearrange("p (v m) -> p v m", m=5),
                featc[:, 32:48]
                .rearrange("p (v o) -> p v o", o=1)
                .to_broadcast([T, 16, 5]),
                sh_all[:, t, 4:9]
                .rearrange("p (o m) -> p o m", o=1)
                .to_broadcast([T, 16, 5]),
                op=OP.mult,
            )
            msg_by_t[t] = msg

        def emit_tile_scatter(t):
            msg = msg_by_t.pop(t)
            nc.sync.dma_start(out_d.ap()[:, t, :], msg[:])

        emit_mm1(0)
        for t in range(NT):
            # tail work for old tiles first: all inputs are >=1 iteration old,
            # so no engine stalls at the head of its in-order stream
            if t >= 2:
                emit_tile_reduce(t - 2)
                emit_tile_msg(t - 2)
            emit_tile_head(t)
            if (t + 1) % 4 == 0 and t + 1 < NT:
                emit_mm1((t + 1) // 4)
            if t >= 3:
                emit_tile_scatter(t - 3)
            if t == 1:
                emit_sh_part2()
        for t in range(NT - 2, NT):
            emit_tile_reduce(t)
            emit_tile_msg(t)
        for t in range(NT - 3, NT):
            emit_tile_scatter(t)

        # output: chunked DMAs, small final chunk so the tail drains fast
        pass

    nc.compile()
    return nc


def _get_program():
    global _PROG
    if _PROG is None:
        _PROG = _build_program()
    return _PROG


def _prep_core(c, h_src, edge_vec, edge_scalars, src_idx, dst_idx, inv_cnt):
    """Shard + sort + gather + one-hot build for one core (partition-major)."""
    import ml_dtypes

    bf = ml_dtypes.bfloat16
    lo, hi = c * EC, (c + 1) * EC
    d = dst_idx[lo:hi]
    order = np.argsort(d, kind="stable")
    d_s = d[order]
    s_s = src_idx[lo:hi][order]

    esT = np.zeros((ESD, EP), np.float32)
    esT[:, :EC] = edge_scalars[lo:hi][order].T
    # x rows pre-scaled by 1/cnt[dst] (folds the scatter-mean divide); pads 0
    x = np.zeros((EP, MUL_SRC), np.float32)
    x[:EC] = h_src[s_s] * inv_cnt[d_s][:, None]
    xrep = np.concatenate([x.T, x.T], axis=0)  # [128, EP]
    ev = np.zeros((EP, 3), np.float32)
    ev[:EC] = edge_vec[lo:hi][order]
    ev[EC:, 0] = 1.0

    labels = np.full(EP, N_DST, np.int64)
    labels[:EC] = d_s

    # partition-major device layouts: [p, t, ...] = row t*T + p
    def pmaj(a):
        return np.ascontiguousarray(a.reshape(NT, T, -1).transpose(1, 0, 2))

    return (
        {
            "esT": esT.astype(bf),
            "xrep": np.ascontiguousarray(xrep).astype(bf),
            "ev": pmaj(ev),
        },
        labels,
    )


def kernel(**inputs):
    import ml_dtypes

    from concourse import bass_utils

    bf = ml_dtypes.bfloat16

    h_src = np.asarray(inputs["h_src"], np.float32)
    edge_vec = np.asarray(inputs["edge_vec"], np.float32)
    edge_scalars = np.asarray(inputs["edge_scalars"], np.float32)
    W1 = np.asarray(inputs["W1"], np.float32)
    b1 = np.asarray(inputs["b1"], np.float32)
    W2 = np.asarray(inputs["W2"], np.float32)
    b2 = np.asarray(inputs["b2"], np.float32)
    src_idx = np.asarray(inputs["src_idx"]).astype(np.int64)
    dst_idx = np.asarray(inputs["dst_idx"]).astype(np.int64)
    n_dst = int(inputs["n_dst"])
    assert n_dst == N_DST

    nc = _get_program()

    cnt = np.bincount(dst_idx, minlength=N_DST)
    inv_cnt = (1.0 / np.maximum(cnt, 1)).astype(np.float32)

    # weights in (l,v,u) column order, pre-scaled by 1/sqrt(64)
    scale = 1.0 / np.sqrt(MUL_SRC)
    W2p = (
        W2.reshape(HID, N_PATHS, MUL_SRC, MUL_DST).transpose(0, 1, 3, 2) * scale
    ).reshape(HID, WCOLS)
    # bias path: feat += x @ b2r with b2r = b2 permuted to (u, l, v).  The
    # device does it as matmul(lhsT=xrep, rhs=b2r-padded); xrep rows 64:128
    # duplicate rows 0:64, so the rhs pads rows 64:128 with zeros.
    b2r = np.zeros((HID, NLV), np.float32)
    b2r[0:MUL_SRC] = (
        b2.reshape(N_PATHS, MUL_SRC, MUL_DST).transpose(1, 0, 2) * scale
    ).reshape(MUL_SRC, NLV)

    # E2: rows 0:64 -> col 0, rows 64:128 -> col 1
    e2 = np.zeros((HID, 2), np.float32)
    e2[0:MUL_SRC, 0] = 1.0
    e2[MUL_SRC:, 1] = 1.0

    shc = np.broadcast_to(
        np.array(
            [SQ3, SQ3, SQ3, SQ15, SQ15, 0.5 * SQ5, SQ15, 0.5 * SQ15], np.float32
        ),
        (T, 8),
    ).copy()

    shared = {
        "W1": np.ascontiguousarray(W1).astype(bf),
        "b1": b1.reshape(HID, 1).astype(np.float32),
        "W2p": W2p.astype(bf),
        "b2r": b2r.astype(bf),
        "e2": e2.astype(bf),
        "shc": shc,
    }

    in_maps = []
    labels_all = []
    for c in range(N_CORES):
        m, labels = _prep_core(
            c, h_src, edge_vec, edge_scalars, src_idx, dst_idx, inv_cnt
        )
        m.update(shared)
        in_maps.append(m)
        labels_all.append(labels)

    import time

    t0 = time.perf_counter()
    res = bass_utils.run_bass_kernel_spmd(nc, in_maps, core_ids=list(range(N_CORES)))
    t1 = time.perf_counter()
    kernel.last_device_wall_s = t1 - t0

    # outp is [T, NT, 144] partition-major; row (t, p) lives at [p, t, :]
    rows = np.concatenate(
        [
            res.results[c]["outp"]
            .astype(np.float32)
            .transpose(1, 0, 2)
            .reshape(NT * T, 144)
            for c in range(N_CORES)
        ],
        axis=0,
    )
    labels = np.concatenate(labels_all)

    order = np.argsort(labels, kind="stable")
    lab_s = labels[order]
    rows_s = rows[order]
    starts = np.concatenate(([0], np.flatnonzero(np.diff(lab_s)) + 1))
    sums = np.add.reduceat(rows_s, starts, axis=0)
    out = np.zeros((N_DST + 1, 144), np.float32)
    out[lab_s[starts]] = sums
    return out[:N_DST]


# revision 5
# speedup vs baseline: 1.0679x; 1.0040x over previous
"""Trainium2 Bass kernel for EquivariantTPConv (gnn_message_passing), v5.

Computation per edge e:
  sh  = SH_l012(edge_vec[e])                                  # [9]
  w   = (silu(edge_scalars[e] @ W1 + b1) @ W2 + b2)           # [3*64*16]
  x   = h_src[src_idx[e]]                                     # [64]
  feat[l,v] = sum_u x[u] * w[l,u,v] / 8                       # [3,16]
  msg = concat_l (feat[:,l,:,None] * sh_l[None,:])            # [144]
  out[d] = mean over {e: dst_idx[e]==d} msg[e]                # [n_dst,144]

v2 strategy (vs the v1 elementwise pipeline):
  - mm2 is emitted TRANSPOSED: 24 matmuls per tile with lhsT = W2p column
    block [128h, 128] and rhs = h2 [128h, 128e], giving wT[(l,v,u)-slice,
    e] in PSUM.  Each 128-partition block holds 2 (l,v) pairs x 64 u.
  - PSUM->SBUF bf16 cast of wT is split between ScalarE and GpSimd.
  - VectorE multiplies by xrep (x^T duplicated on 128 partitions, host-
    prepared, pre-scaled by 1/cnt[dst]) in 2x bf16 mode -> prodT.
  - The u-reduction is 24 tiny PE matmuls: lhsT = prodT block [128, 128e],
    rhs = E2 [128, 2] block-ones -> feat[T, 2] each, written into a shared
    feat PSUM region.  The b2 bias lands there too via one extra matmul
    with lhsT = xrep and rhs = b2 rows (b2 pre-divided into the two
    64-row halves).
  - msg outer products with SH on GpSimd read feat straight from PSUM;
    scatter uses the one-hot matmul; 1/cnt is pre-folded into xrep so the
    eviction is a plain copy on VectorE.
  - Host unshard: segment-sum of slot rows across tiles/cores (unchanged).
"""

import sys

for _p in ("/opt/trn_rl_repo", "/root/.axon_site/_ro/trn_rl_repo"):
    if _p not in sys.path:
        sys.path.append(_p)

import numpy as np

MUL_SRC = 64
MUL_DST = 16
N_PATHS = 3
SQ3 = 3.0 ** 0.5
SQ5 = 5.0 ** 0.5
SQ15 = 15.0 ** 0.5

N_CORES = 8
E_TOT = 50000
N_SRC = 10000
N_DST = 10000
ESD = 32
HID = 128
WCOLS = N_PATHS * MUL_DST * MUL_SRC  # 3072 (perm layout (l,v,u))
NLV = N_PATHS * MUL_DST  # 48

T = 128  # edges per tile
BLK = 512  # edges per full MM1 block (last block is a 128-edge tail)
EC = E_TOT // N_CORES  # 6250 edges per core
NT = (EC + T - 1) // T  # 49 tiles
EP = NT * T  # 6272 padded edges per core
NB = (EP + BLK - 1) // BLK  # 13 blocks, last one covers a single tile

NSB = WCOLS // T  # 24 (l,v,u) column blocks of 128 per tile
NSH = NSB // 2  # 12 blocks per half
HW = NSH * T  # 1536 cols per half

# per half: blocks 0..7 (1024 cols) go ScalarE-cast + VectorE-mult via pwS;
# blocks 8..11 (512 cols) go GpSimd fused cast*mult straight from pwP PSUM.
NS_S = 8  # pwS blocks per half
CS = NS_S * T  # 1024
NS_P = NSH - NS_S  # 4
CP = NS_P * T  # 512

_PROG = None  # cached compiled program


def _build_program():
    from contextlib import ExitStack

    import concourse.tile as tile
    from concourse import bacc, mybir

    f32 = mybir.dt.float32
    bf16 = mybir.dt.bfloat16
    AF = mybir.ActivationFunctionType
    OP = mybir.AluOpType
    AX = mybir.AxisListType

    nc = bacc.Bacc(
        "TRN2",
        target_bir_lowering=False,
        debug=False,
        enable_asserts=False,
        num_devices=N_CORES,
    )

    # DRAM inputs; all big per-core tensors are partition-major on the host.
    esT_d = nc.dram_tensor("esT", [ESD, EP], bf16, kind="ExternalInput")
    xrep_d = nc.dram_tensor("xrep", [HID, EP], bf16, kind="ExternalInput")
    ev_d = nc.dram_tensor("ev", [T, NT, 3], f32, kind="ExternalInput")
    W1_d = nc.dram_tensor("W1", [ESD, HID], bf16, kind="ExternalInput")
    b1_d = nc.dram_tensor("b1", [HID, 1], f32, kind="ExternalInput")
    W2_d = nc.dram_tensor("W2p", [HID, WCOLS], bf16, kind="ExternalInput")
    b2_d = nc.dram_tensor("b2r", [HID, NLV], bf16, kind="ExternalInput")
    e2_d = nc.dram_tensor("e2", [HID, 2], bf16, kind="ExternalInput")
    shc_d = nc.dram_tensor("shc", [T, 8], f32, kind="ExternalInput")
    out_d = nc.dram_tensor("outp", [T, NT, 144], bf16, kind="ExternalOutput")

    with ExitStack() as ctx:
        tc = ctx.enter_context(tile.TileContext(nc))

        const = ctx.enter_context(tc.tile_pool(name="const", bufs=1))
        shp = ctx.enter_context(tc.tile_pool(name="shp", bufs=1))
        h2pool = ctx.enter_context(tc.tile_pool(name="h2pool", bufs=2))
        wbp = ctx.enter_context(tc.tile_pool(name="wbp", bufs=3))
        prp = ctx.enter_context(tc.tile_pool(name="prp", bufs=3))
        msgp = ctx.enter_context(tc.tile_pool(name="msgp", bufs=10))
        ph1 = ctx.enter_context(tc.tile_pool(name="ph1", bufs=1, space="PSUM"))
        pwt = ctx.enter_context(tc.tile_pool(name="pwt", bufs=2, space="PSUM"))
        pfs = ctx.enter_context(tc.tile_pool(name="pfs", bufs=1, space="PSUM"))

        # ---- resident loads; SP carries the edge streams, GpSimd's queue
        # carries the weights, ordered by first use ----
        W1s = const.tile([ESD, HID], bf16)
        nc.gpsimd.dma_start(W1s[:], W1_d.ap())
        b1s = const.tile([HID, 1], f32)
        nc.gpsimd.dma_start(b1s[:], b1_d.ap())
        W2s = const.tile([HID, WCOLS], bf16)
        nc.gpsimd.dma_start(W2s[:, 0:1024], W2_d.ap()[:, 0:1024])
        nc.gpsimd.dma_start(W2s[:, 1024:], W2_d.ap()[:, 1024:])
        b2s = const.tile([HID, NLV], bf16)
        nc.gpsimd.dma_start(b2s[:], b2_d.ap())
        e2s = const.tile([HID, 2], bf16)
        nc.gpsimd.dma_start(e2s[:], e2_d.ap())
        shcs = const.tile([T, 8], f32)
        nc.gpsimd.dma_start(shcs[:], shc_d.ap())

        es_all = const.tile([ESD, EP], bf16)
        nc.sync.dma_start(es_all[:, 0:BLK], esT_d.ap()[:, 0:BLK])
        xrep_all = const.tile([HID, EP], bf16)
        nc.sync.dma_start(xrep_all[:, 0 : 4 * T], xrep_d.ap()[:, 0 : 4 * T])
        ev_all = const.tile([T, NT, 3], f32)
        nc.sync.dma_start(ev_all[:], ev_d.ap())

        nc.sync.dma_start(es_all[:, BLK:], esT_d.ap()[:, BLK:])
        nc.sync.dma_start(xrep_all[:, 4 * T :], xrep_d.ap()[:, 4 * T :])
        negone = const.tile([T, 1], f32)
        nc.vector.memset(negone[:], -1.0)


        # ---- SH prologue: all edges at once, [128, NT, k] layouts ----
        sq_all = shp.tile([T, NT, 3], f32)
        nc.vector.tensor_tensor(sq_all[:], ev_all[:], ev_all[:], op=OP.mult)
        r2_all = shp.tile([T, NT], f32)
        nc.vector.tensor_reduce(r2_all[:], sq_all[:], axis=AX.X, op=OP.add)
        rn_all = shp.tile([T, NT], f32)
        nc.scalar.activation(rn_all[:], r2_all[:], AF.Sqrt)

        def bc(ap_, shape):
            return ap_.to_broadcast(shape)

        sh_all = shp.tile([T, NT, 9], f32)

        def emit_sh_part2():
            inv_all = shp.tile([T, NT], f32)
            nc.vector.reciprocal(inv_all[:], rn_all[:])
            inv2_all = shp.tile([T, NT], f32)
            nc.vector.tensor_tensor(inv2_all[:], inv_all[:], inv_all[:], op=OP.mult)
            i1 = inv_all[:].rearrange("p (t o) -> p t o", o=1)
            i2 = inv2_all[:].rearrange("p (t o) -> p t o", o=1)
            nc.vector.tensor_tensor(
                sh_all[:, :, 1:4], ev_all[:], bc(i1, [T, NT, 3]), op=OP.mult
            )
            pq_all = shp.tile([T, NT, 2], f32)
            nc.vector.tensor_tensor(
                pq_all[:], ev_all[:, :, 0:2], ev_all[:, :, 1:3], op=OP.mult
            )
            nc.vector.tensor_tensor(
                sh_all[:, :, 4:6], pq_all[:], bc(i2, [T, NT, 2]), op=OP.mult
            )
            t6_all = shp.tile([T, NT], f32)
            nc.vector.tensor_tensor(
                t6_all[:].rearrange("p (t o) -> p t o", o=1),
                sq_all[:, :, 2:3],
                i2,
                op=OP.mult,
            )
            nc.scalar.activation(
                sh_all[:, :, 6], t6_all[:], AF.Identity, bias=negone[:, 0:1], scale=3.0
            )
            xz_all = shp.tile([T, NT, 1], f32)
            nc.vector.tensor_tensor(
                xz_all[:], ev_all[:, :, 0:1], ev_all[:, :, 2:3], op=OP.mult
            )
            nc.vector.tensor_tensor(sh_all[:, :, 7:8], xz_all[:], i2, op=OP.mult)
            d2_all = shp.tile([T, NT, 1], f32)
            nc.vector.tensor_tensor(
                d2_all[:], sq_all[:, :, 0:1], sq_all[:, :, 1:2], op=OP.subtract
            )
            nc.vector.tensor_tensor(sh_all[:, :, 8:9], d2_all[:], i2, op=OP.mult)
            shc3 = shcs[:].rearrange("p (o c) -> p o c", o=1)
            nc.vector.tensor_tensor(
                sh_all[:, :, 1:9], sh_all[:, :, 1:9], bc(shc3, [T, NT, 8]), op=OP.mult
            )

        # ---- main pipeline ----
        # stage A(t): mm2 halves -> pwt PSUM; casts (S/P); mult (V) -> prodT
        # stage B(t-1): bias-mm + 24 reduce-mms -> feat region of fp
        #               gpsimd msg outers from feat PSUM; l0 copy
        # stage C(t-2): scatter-mm -> ps region of fp; V evict -> ob_all
        prodT_by_t = {}
        feat_by_t = {}
        msg_by_t = {}

        # single PSUM bank shared by feat [0:48] and the scatter out [64:208]
        fp = pfs.tile([T, 512], f32, tag="fp", name="fp")

        def emit_mm1(b):
            nbt = min(4, NT - b * 4)
            bw = nbt * T
            h1 = ph1.tile([HID, BLK], f32, tag="h1", name=f"h1_{b}")
            nc.tensor.matmul(
                h1[:, 0:bw],
                W1s[:],
                es_all[:, b * BLK : b * BLK + bw],
                start=True,
                stop=True,
            )
            h2 = h2pool.tile([HID, BLK], bf16, tag="h2", name=f"h2_{b}")
            nc.scalar.activation(h2[:, 0:bw], h1[:, 0:bw], AF.Silu, bias=b1s[:, 0:1])
            emit_mm1.h2_by_b[b] = h2

        emit_mm1.h2_by_b = {}

        def emit_tile_head(t):
            b, q = divmod(t, 4)
            h2 = emit_mm1.h2_by_b[b]
            if q == 3 and b - 1 in emit_mm1.h2_by_b:
                del emit_mm1.h2_by_b[b - 1]
            prodT = prp.tile([HID, WCOLS], bf16, tag="prodT", name=f"prodT{t}")
            xe = xrep_all[:, t * T : (t + 1) * T]
            xb1 = xe.rearrange("p (o e) -> p o e", o=1)
            rhs = h2[:, q * T : (q + 1) * T]
            # all pwS matmuls first so the S casts never wait behind pwP work
            pwS_h, pwP_h = [], []
            for hh in range(2):
                pwS = pwt.tile([HID, CS], f32, tag="pwS", name=f"pwS{t}_{hh}", bufs=2)
                pwS_h.append(pwS)
                for j in range(NS_S):
                    s = hh * NSH + j
                    nc.tensor.matmul(
                        pwS[:, j * T : (j + 1) * T],
                        W2s[:, s * T : (s + 1) * T],
                        rhs,
                        start=True,
                        stop=True,
                    )
            for hh in range(2):
                pwP = pwt.tile([HID, CP], f32, tag="pwP", name=f"pwP{t}_{hh}", bufs=2)
                pwP_h.append(pwP)
                for j in range(NS_P):
                    s = hh * NSH + NS_S + j
                    nc.tensor.matmul(
                        pwP[:, j * T : (j + 1) * T],
                        W2s[:, s * T : (s + 1) * T],
                        rhs,
                        start=True,
                        stop=True,
                    )
            for hh in range(2):
                wb = wbp.tile([HID, CS], bf16, tag="wb", name=f"wb{t}_{hh}")
                nc.scalar.activation(wb[:], pwS_h[hh][:], AF.Copy)
                # GpSimd fused cast*mult straight from PSUM for blocks 8..11
                pp = prodT[:, hh * HW + CS : (hh + 1) * HW].rearrange(
                    "p (s e) -> p s e", e=T
                )
                nc.vector.tensor_tensor(
                    pp,
                    pwP_h[hh][:].rearrange("p (s e) -> p s e", e=T),
                    xb1.to_broadcast([HID, NS_P, T]),
                    op=OP.mult,
                )
                # VectorE 2x bf16 multiply for the ScalarE-cast blocks
                po = prodT[:, hh * HW : hh * HW + CS - 384].rearrange(
                    "p (s e) -> p s e", e=T
                )
                nc.vector.tensor_tensor(
                    po,
                    wb[:, 0 : CS - 384].rearrange("p (s e) -> p s e", e=T),
                    xb1.to_broadcast([HID, NS_S - 3, T]),
                    op=OP.mult,
                )
                nc.gpsimd.tensor_tensor(
                    prodT[:, hh * HW + CS - 384 : hh * HW + CS].rearrange(
                        "p (s e) -> p s e", e=T
                    ),
                    wb[:, CS - 384 : CS].rearrange("p (s e) -> p s e", e=T),
                    xb1.to_broadcast([HID, 3, T]),
                    op=OP.mult,
                )
            prodT_by_t[t] = prodT

        def emit_tile_reduce(t):
            prodT = prodT_by_t.pop(t)
            feat = fp[:, 256 * (t % 2) : 256 * (t % 2) + NLV]
            # bias: lhsT = xrep (full 128 partitions), rhs = b2 rows (/1 in
            # rows 0:64, zeros in 64:128)
            nc.tensor.matmul(
                feat, xrep_all[:, t * T : (t + 1) * T], b2s[:], start=True, stop=False
            )
            off = 256 * (t % 2)
            for s in range(NSB):
                nc.tensor.matmul(
                    fp[:, off + 2 * s : off + 2 * s + 2],
                    prodT[:, s * T : (s + 1) * T],
                    e2s[:],
                    start=False,
                    stop=True,
                )
            feat_by_t[t] = feat

        def emit_tile_msg(t):
            feat = feat_by_t.pop(t)
            msg = msgp.tile([T, 144], bf16, tag="msg", name=f"msg{t}")
            featc = msgp.tile([T, NLV], bf16, tag="featc", name=f"featc{t}")
            nc.vector.tensor_copy(featc[:], feat)
            nc.gpsimd.tensor_copy(msg[:, 0:16], featc[:, 0:16])
            nc.gpsimd.tensor_tensor(
                msg[:, 16:64].rearrange("p (v m) -> p v m", m=3),
                featc[:, 16:32]
                .rearrange("p (v o) -> p v o", o=1)
                .to_broadcast([T, 16, 3]),
                sh_all[:, t, 1:4]
                .rearrange("p (o m) -> p o m", o=1)
                .to_broadcast([T, 16, 3]),
                op=OP.mult,
            )
            nc.gpsimd.tensor_tensor(
                msg[:, 64:144].rearrange("p (v m) -> p v m", m=5),
                featc[:, 32:48]
                .rearrange("p (v o) -> p v o", o=1)
                .to_broadcast([T, 16, 5]),
                sh_all[:, t, 4:9]
                .rearrange("p (o m) -> p o m", o=1)
                .to_broadcast([T, 16, 5]),
                op=OP.mult,
            )
            msg_by_t[t] = msg

        def emit_tile_scatter(t):
            msg = msg_by_t.pop(t)
            nc.sync.dma_start(out_d.ap()[:, t, :], msg[:])

        emit_mm1(0)
        for t in range(NT):
            # tail work for old tiles first: all inputs are >=1 iteration old,
            # so no engine stalls at the head of its in-order stream
            if t >= 3:
                emit_tile_reduce(t - 3)
                emit_tile_msg(t - 3)
            emit_tile_head(t)
            if (t + 2) % 4 == 0 and t + 2 < NT:
                emit_mm1((t + 2) // 4)
            if t >= 4:
                emit_tile_scatter(t - 4)
            if t == 1:
                emit_sh_part2()
        for t in range(NT - 3, NT):
            emit_tile_reduce(t)
            emit_tile_msg(t)
        for t in range(NT - 4, NT):
            emit_tile_scatter(t)

        # output: chunked DMAs, small final chunk so the tail drains fast
        pass

    nc.compile()
    return nc


def _get_program():
    global _PROG
    if _PROG is None:
        _PROG = _build_program()
    return _PROG


def _prep_core(c, h_src, edge_vec, edge_scalars, src_idx, dst_idx, inv_cnt):
    """Shard + sort + gather + one-hot build for one core (partition-major)."""
    import ml_dtypes

    bf = ml_dtypes.bfloat16
    lo, hi = c * EC, (c + 1) * EC
    d = dst_idx[lo:hi]
    order = np.argsort(d, kind="stable")
    d_s = d[order]
    s_s = src_idx[lo:hi][order]

    esT = np.zeros((ESD, EP), np.float32)
    esT[:, :EC] = edge_scalars[lo:hi][order].T
    # x rows pre-scaled by 1/cnt[dst] (folds the scatter-mean divide); pads 0
    x = np.zeros((EP, MUL_SRC), np.float32)
    x[:EC] = h_src[s_s] * inv_cnt[d_s][:, None]
    xrep = np.concatenate([x.T, x.T], axis=0)  # [128, EP]
    ev = np.zeros((EP, 3), np.float32)
    ev[:EC] = edge_vec[lo:hi][order]
    ev[EC:, 0] = 1.0

    labels = np.full(EP, N_DST, np.int64)
    labels[:EC] = d_s

    # partition-major device layouts: [p, t, ...] = row t*T + p
    def pmaj(a):
        return np.ascontiguousarray(a.reshape(NT, T, -1).transpose(1, 0, 2))

    return (
        {
            "esT": esT.astype(bf),
            "xrep": np.ascontiguousarray(xrep).astype(bf),
            "ev": pmaj(ev),
        },
        labels,
    )


def kernel(**inputs):
    import ml_dtypes

    from concourse import bass_utils

    bf = ml_dtypes.bfloat16

    h_src = np.asarray(inputs["h_src"], np.float32)
    edge_vec = np.asarray(inputs["edge_vec"], np.float32)
    edge_scalars = np.asarray(inputs["edge_scalars"], np.float32)
    W1 = np.asarray(inputs["W1"], np.float32)
    b1 = np.asarray(inputs["b1"], np.float32)
    W2 = np.asarray(inputs["W2"], np.float32)
    b2 = np.asarray(inputs["b2"], np.float32)
    src_idx = np.asarray(inputs["src_idx"]).astype(np.int64)
    dst_idx = np.asarray(inputs["dst_idx"]).astype(np.int64)
    n_dst = int(inputs["n_dst"])
    assert n_dst == N_DST

    nc = _get_program()

    cnt = np.bincount(dst_idx, minlength=N_DST)
    inv_cnt = (1.0 / np.maximum(cnt, 1)).astype(np.float32)

    # weights in (l,v,u) column order, pre-scaled by 1/sqrt(64)
    scale = 1.0 / np.sqrt(MUL_SRC)
    W2p = (
        W2.reshape(HID, N_PATHS, MUL_SRC, MUL_DST).transpose(0, 1, 3, 2) * scale
    ).reshape(HID, WCOLS)
    # bias path: feat += x @ b2r with b2r = b2 permuted to (u, l, v).  The
    # device does it as matmul(lhsT=xrep, rhs=b2r-padded); xrep rows 64:128
    # duplicate rows 0:64, so the rhs pads rows 64:128 with zeros.
    b2r = np.zeros((HID, NLV), np.float32)
    b2r[0:MUL_SRC] = (
        b2.reshape(N_PATHS, MUL_SRC, MUL_DST).transpose(1, 0, 2) * scale
    ).reshape(MUL_SRC, NLV)

    # E2: rows 0:64 -> col 0, rows 64:128 -> col 1
    e2 = np.zeros((HID, 2), np.float32)
    e2[0:MUL_SRC, 0] = 1.0
    e2[MUL_SRC:, 1] = 1.0

    shc = np.broadcast_to(
        np.array(
            [SQ3, SQ3, SQ3, SQ15, SQ15, 0.5 * SQ5, SQ15, 0.5 * SQ15], np.float32
        ),
        (T, 8),
    ).copy()

    shared = {
        "W1": np.ascontiguousarray(W1).astype(bf),
        "b1": b1.reshape(HID, 1).astype(np.float32),
        "W2p": W2p.astype(bf),
        "b2r": b2r.astype(bf),
        "e2": e2.astype(bf),
        "shc": shc,
    }

    in_maps = []
    labels_all = []
    for c in range(N_CORES):
        m, labels = _prep_core(
            c, h_src, edge_vec, edge_scalars, src_idx, dst_idx, inv_cnt
        )
        m.update(shared)
        in_maps.append(m)
        labels_all.append(labels)

    import time

    t0 = time.perf_counter()
    res = bass_utils.run_bass_kernel_spmd(nc, in_maps, core_ids=list(range(N_CORES)))
    t1 = time.perf_counter()
    kernel.last_device_wall_s = t1 - t0

    # outp is [T, NT, 144] partition-major; row (t, p) lives at [p, t, :]
    rows = np.concatenate(
        [
            res.results[c]["outp"]
            .astype(np.float32)
            .transpose(1, 0, 2)
            .reshape(NT * T, 144)
            for c in range(N_CORES)
        ],
        axis=0,
    )
    labels = np.concatenate(labels_all)

    order = np.argsort(labels, kind="stable")
    lab_s = labels[order]
    rows_s = rows[order]
    starts = np.concatenate(([0], np.flatnonzero(np.diff(lab_s)) + 1))
    sums = np.add.reduceat(rows_s, starts, axis=0)
    out = np.zeros((N_DST + 1, 144), np.float32)
    out[lab_s[starts]] = sums
    return out[:N_DST]


# revision 6
# speedup vs baseline: 1.0941x; 1.0245x over previous
"""Trainium2 Bass kernel for EquivariantTPConv (gnn_message_passing), v5.

Computation per edge e:
  sh  = SH_l012(edge_vec[e])                                  # [9]
  w   = (silu(edge_scalars[e] @ W1 + b1) @ W2 + b2)           # [3*64*16]
  x   = h_src[src_idx[e]]                                     # [64]
  feat[l,v] = sum_u x[u] * w[l,u,v] / 8                       # [3,16]
  msg = concat_l (feat[:,l,:,None] * sh_l[None,:])            # [144]
  out[d] = mean over {e: dst_idx[e]==d} msg[e]                # [n_dst,144]

v2 strategy (vs the v1 elementwise pipeline):
  - mm2 is emitted TRANSPOSED: 24 matmuls per tile with lhsT = W2p column
    block [128h, 128] and rhs = h2 [128h, 128e], giving wT[(l,v,u)-slice,
    e] in PSUM.  Each 128-partition block holds 2 (l,v) pairs x 64 u.
  - PSUM->SBUF bf16 cast of wT is split between ScalarE and GpSimd.
  - VectorE multiplies by xrep (x^T duplicated on 128 partitions, host-
    prepared, pre-scaled by 1/cnt[dst]) in 2x bf16 mode -> prodT.
  - The u-reduction is 24 tiny PE matmuls: lhsT = prodT block [128, 128e],
    rhs = E2 [128, 2] block-ones -> feat[T, 2] each, written into a shared
    feat PSUM region.  The b2 bias lands there too via one extra matmul
    with lhsT = xrep and rhs = b2 rows (b2 pre-divided into the two
    64-row halves).
  - msg outer products with SH on GpSimd read feat straight from PSUM;
    scatter uses the one-hot matmul; 1/cnt is pre-folded into xrep so the
    eviction is a plain copy on VectorE.
  - Host unshard: segment-sum of slot rows across tiles/cores (unchanged).
"""

import sys

for _p in ("/opt/trn_rl_repo", "/root/.axon_site/_ro/trn_rl_repo"):
    if _p not in sys.path:
        sys.path.append(_p)

import numpy as np

MUL_SRC = 64
MUL_DST = 16
N_PATHS = 3
SQ3 = 3.0 ** 0.5
SQ5 = 5.0 ** 0.5
SQ15 = 15.0 ** 0.5

N_CORES = 8
E_TOT = 50000
N_SRC = 10000
N_DST = 10000
ESD = 32
HID = 128
WCOLS = N_PATHS * MUL_DST * MUL_SRC  # 3072 (perm layout (l,v,u))
NLV = N_PATHS * MUL_DST  # 48

T = 128  # edges per tile
BLK = 512  # edges per full MM1 block (last block is a 128-edge tail)
EC = E_TOT // N_CORES  # 6250 edges per core
NT = (EC + T - 1) // T  # 49 tiles
EP = NT * T  # 6272 padded edges per core
NB = (EP + BLK - 1) // BLK  # 13 blocks, last one covers a single tile

NSB = WCOLS // T  # 24 (l,v,u) column blocks of 128 per tile
NSH = NSB // 2  # 12 blocks per half
HW = NSH * T  # 1536 cols per half

# per half: blocks 0..7 (1024 cols) go ScalarE-cast + VectorE-mult via pwS;
# blocks 8..11 (512 cols) go GpSimd fused cast*mult straight from pwP PSUM.
NS_S = 8  # pwS blocks per half
CS = NS_S * T  # 1024
NS_P = NSH - NS_S  # 4
CP = NS_P * T  # 512

_PROG = None  # cached compiled program


def _build_program():
    from contextlib import ExitStack

    import concourse.tile as tile
    from concourse import bacc, mybir

    f32 = mybir.dt.float32
    bf16 = mybir.dt.bfloat16
    AF = mybir.ActivationFunctionType
    OP = mybir.AluOpType
    AX = mybir.AxisListType

    nc = bacc.Bacc(
        "TRN2",
        target_bir_lowering=False,
        debug=False,
        enable_asserts=False,
        num_devices=N_CORES,
    )

    # DRAM inputs; all big per-core tensors are partition-major on the host.
    esT_d = nc.dram_tensor("esT", [ESD, EP], bf16, kind="ExternalInput")
    xrep_d = nc.dram_tensor("xrep", [HID, EP], bf16, kind="ExternalInput")
    ev_d = nc.dram_tensor("ev", [T, NT, 3], f32, kind="ExternalInput")
    W1_d = nc.dram_tensor("W1", [ESD, HID], bf16, kind="ExternalInput")
    b1_d = nc.dram_tensor("b1", [HID, 1], f32, kind="ExternalInput")
    W2_d = nc.dram_tensor("W2p", [HID, WCOLS], bf16, kind="ExternalInput")
    b2_d = nc.dram_tensor("b2r", [HID, NLV], bf16, kind="ExternalInput")
    e2_d = nc.dram_tensor("e2", [HID, 2], bf16, kind="ExternalInput")
    shc_d = nc.dram_tensor("shc", [T, 8], f32, kind="ExternalInput")
    out_d = nc.dram_tensor("outp", [T, NT, 144], bf16, kind="ExternalOutput")

    with ExitStack() as ctx:
        tc = ctx.enter_context(tile.TileContext(nc))

        const = ctx.enter_context(tc.tile_pool(name="const", bufs=1))
        shp = ctx.enter_context(tc.tile_pool(name="shp", bufs=1))
        h2pool = ctx.enter_context(tc.tile_pool(name="h2pool", bufs=2))
        wbp = ctx.enter_context(tc.tile_pool(name="wbp", bufs=3))
        prp = ctx.enter_context(tc.tile_pool(name="prp", bufs=3))
        msgp = ctx.enter_context(tc.tile_pool(name="msgp", bufs=10))
        ph1 = ctx.enter_context(tc.tile_pool(name="ph1", bufs=1, space="PSUM"))
        pwt = ctx.enter_context(tc.tile_pool(name="pwt", bufs=2, space="PSUM"))
        pfs = ctx.enter_context(tc.tile_pool(name="pfs", bufs=1, space="PSUM"))

        # ---- resident loads; SP carries the edge streams, GpSimd's queue
        # carries the weights, ordered by first use ----
        W1s = const.tile([ESD, HID], bf16)
        nc.gpsimd.dma_start(W1s[:], W1_d.ap())
        b1s = const.tile([HID, 1], f32)
        nc.gpsimd.dma_start(b1s[:], b1_d.ap())
        W2s = const.tile([HID, WCOLS], bf16)
        nc.gpsimd.dma_start(W2s[:, 0:1024], W2_d.ap()[:, 0:1024])
        nc.gpsimd.dma_start(W2s[:, 1024:], W2_d.ap()[:, 1024:])
        b2s = const.tile([HID, NLV], bf16)
        nc.gpsimd.dma_start(b2s[:], b2_d.ap())
        e2s = const.tile([HID, 2], bf16)
        nc.gpsimd.dma_start(e2s[:], e2_d.ap())
        shcs = const.tile([T, 8], f32)
        nc.gpsimd.dma_start(shcs[:], shc_d.ap())

        es_all = const.tile([ESD, EP], bf16)
        nc.sync.dma_start(es_all[:, 0:BLK], esT_d.ap()[:, 0:BLK])
        xrep_all = const.tile([HID, EP], bf16)
        nc.sync.dma_start(xrep_all[:, 0 : 4 * T], xrep_d.ap()[:, 0 : 4 * T])
        ev_all = const.tile([T, NT, 3], f32)
        nc.gpsimd.dma_start(ev_all[:], ev_d.ap())

        nc.sync.dma_start(es_all[:, BLK:], esT_d.ap()[:, BLK:])
        nc.sync.dma_start(xrep_all[:, 4 * T :], xrep_d.ap()[:, 4 * T :])
        negone = const.tile([T, 1], f32)
        nc.vector.memset(negone[:], -1.0)


        # ---- SH prologue: all edges at once, [128, NT, k] layouts ----
        sq_all = shp.tile([T, NT, 3], f32)
        nc.vector.tensor_tensor(sq_all[:], ev_all[:], ev_all[:], op=OP.mult)
        r2_all = shp.tile([T, NT], f32)
        nc.vector.tensor_reduce(r2_all[:], sq_all[:], axis=AX.X, op=OP.add)
        rn_all = shp.tile([T, NT], f32)
        nc.scalar.activation(rn_all[:], r2_all[:], AF.Sqrt)

        def bc(ap_, shape):
            return ap_.to_broadcast(shape)

        sh_all = shp.tile([T, NT, 9], f32)

        def emit_sh_part2():
            inv_all = shp.tile([T, NT], f32)
            nc.vector.reciprocal(inv_all[:], rn_all[:])
            inv2_all = shp.tile([T, NT], f32)
            nc.vector.tensor_tensor(inv2_all[:], inv_all[:], inv_all[:], op=OP.mult)
            i1 = inv_all[:].rearrange("p (t o) -> p t o", o=1)
            i2 = inv2_all[:].rearrange("p (t o) -> p t o", o=1)
            nc.vector.tensor_tensor(
                sh_all[:, :, 1:4], ev_all[:], bc(i1, [T, NT, 3]), op=OP.mult
            )
            pq_all = shp.tile([T, NT, 2], f32)
            nc.vector.tensor_tensor(
                pq_all[:], ev_all[:, :, 0:2], ev_all[:, :, 1:3], op=OP.mult
            )
            nc.vector.tensor_tensor(
                sh_all[:, :, 4:6], pq_all[:], bc(i2, [T, NT, 2]), op=OP.mult
            )
            t6_all = shp.tile([T, NT], f32)
            nc.vector.tensor_tensor(
                t6_all[:].rearrange("p (t o) -> p t o", o=1),
                sq_all[:, :, 2:3],
                i2,
                op=OP.mult,
            )
            nc.scalar.activation(
                sh_all[:, :, 6], t6_all[:], AF.Identity, bias=negone[:, 0:1], scale=3.0
            )
            xz_all = shp.tile([T, NT, 1], f32)
            nc.vector.tensor_tensor(
                xz_all[:], ev_all[:, :, 0:1], ev_all[:, :, 2:3], op=OP.mult
            )
            nc.vector.tensor_tensor(sh_all[:, :, 7:8], xz_all[:], i2, op=OP.mult)
            d2_all = shp.tile([T, NT, 1], f32)
            nc.vector.tensor_tensor(
                d2_all[:], sq_all[:, :, 0:1], sq_all[:, :, 1:2], op=OP.subtract
            )
            nc.vector.tensor_tensor(sh_all[:, :, 8:9], d2_all[:], i2, op=OP.mult)
            shc3 = shcs[:].rearrange("p (o c) -> p o c", o=1)
            nc.vector.tensor_tensor(
                sh_all[:, :, 1:9], sh_all[:, :, 1:9], bc(shc3, [T, NT, 8]), op=OP.mult
            )

        # ---- main pipeline ----
        # stage A(t): mm2 halves -> pwt PSUM; casts (S/P); mult (V) -> prodT
        # stage B(t-1): bias-mm + 24 reduce-mms -> feat region of fp
        #               gpsimd msg outers from feat PSUM; l0 copy
        # stage C(t-2): scatter-mm -> ps region of fp; V evict -> ob_all
        prodT_by_t = {}
        feat_by_t = {}
        msg_by_t = {}

        # single PSUM bank shared by feat [0:48] and the scatter out [64:208]
        fp = pfs.tile([T, 512], f32, tag="fp", name="fp")

        def emit_mm1(b):
            nbt = min(4, NT - b * 4)
            bw = nbt * T
            h1 = ph1.tile([HID, BLK], f32, tag="h1", name=f"h1_{b}")
            nc.tensor.matmul(
                h1[:, 0:bw],
                W1s[:],
                es_all[:, b * BLK : b * BLK + bw],
                start=True,
                stop=True,
            )
            h2 = h2pool.tile([HID, BLK], bf16, tag="h2", name=f"h2_{b}")
            nc.scalar.activation(h2[:, 0:bw], h1[:, 0:bw], AF.Silu, bias=b1s[:, 0:1])
            emit_mm1.h2_by_b[b] = h2

        emit_mm1.h2_by_b = {}

        def emit_tile_head(t):
            b, q = divmod(t, 4)
            h2 = emit_mm1.h2_by_b[b]
            if q == 3 and b - 1 in emit_mm1.h2_by_b:
                del emit_mm1.h2_by_b[b - 1]
            prodT = prp.tile([HID, WCOLS], bf16, tag="prodT", name=f"prodT{t}")
            xe = xrep_all[:, t * T : (t + 1) * T]
            xb1 = xe.rearrange("p (o e) -> p o e", o=1)
            rhs = h2[:, q * T : (q + 1) * T]
            # all pwS matmuls first so the S casts never wait behind pwP work
            pwS_h, pwP_h = [], []
            for hh in range(2):
                pwS = pwt.tile([HID, CS], f32, tag="pwS", name=f"pwS{t}_{hh}", bufs=2)
                pwS_h.append(pwS)
                for j in range(NS_S):
                    s = hh * NSH + j
                    nc.tensor.matmul(
                        pwS[:, j * T : (j + 1) * T],
                        W2s[:, s * T : (s + 1) * T],
                        rhs,
                        start=True,
                        stop=True,
                    )
            for hh in range(2):
                pwP = pwt.tile([HID, CP], f32, tag="pwP", name=f"pwP{t}_{hh}", bufs=2)
                pwP_h.append(pwP)
                for j in range(NS_P):
                    s = hh * NSH + NS_S + j
                    nc.tensor.matmul(
                        pwP[:, j * T : (j + 1) * T],
                        W2s[:, s * T : (s + 1) * T],
                        rhs,
                        start=True,
                        stop=True,
                    )
            for hh in range(2):
                wb = wbp.tile([HID, CS], bf16, tag="wb", name=f"wb{t}_{hh}")
                nc.scalar.activation(wb[:], pwS_h[hh][:], AF.Copy)
                # GpSimd fused cast*mult straight from PSUM for blocks 8..11
                pp = prodT[:, hh * HW + CS : (hh + 1) * HW].rearrange(
                    "p (s e) -> p s e", e=T
                )
                nc.vector.tensor_tensor(
                    pp,
                    pwP_h[hh][:].rearrange("p (s e) -> p s e", e=T),
                    xb1.to_broadcast([HID, NS_P, T]),
                    op=OP.mult,
                )
                # VectorE 2x bf16 multiply for the ScalarE-cast blocks
                po = prodT[:, hh * HW : hh * HW + CS - 384].rearrange(
                    "p (s e) -> p s e", e=T
                )
                nc.vector.tensor_tensor(
                    po,
                    wb[:, 0 : CS - 384].rearrange("p (s e) -> p s e", e=T),
                    xb1.to_broadcast([HID, NS_S - 3, T]),
                    op=OP.mult,
                )
                nc.gpsimd.tensor_tensor(
                    prodT[:, hh * HW + CS - 384 : hh * HW + CS].rearrange(
                        "p (s e) -> p s e", e=T
                    ),
                    wb[:, CS - 384 : CS].rearrange("p (s e) -> p s e", e=T),
                    xb1.to_broadcast([HID, 3, T]),
                    op=OP.mult,
                )
            prodT_by_t[t] = prodT

        def emit_tile_reduce(t):
            prodT = prodT_by_t.pop(t)
            feat = fp[:, 256 * (t % 2) : 256 * (t % 2) + NLV]
            # bias: lhsT = xrep (full 128 partitions), rhs = b2 rows (/1 in
            # rows 0:64, zeros in 64:128)
            nc.tensor.matmul(
                feat, xrep_all[:, t * T : (t + 1) * T], b2s[:], start=True, stop=False
            )
            off = 256 * (t % 2)
            for s in range(NSB):
                nc.tensor.matmul(
                    fp[:, off + 2 * s : off + 2 * s + 2],
                    prodT[:, s * T : (s + 1) * T],
                    e2s[:],
                    start=False,
                    stop=True,
                )
            feat_by_t[t] = feat

        def emit_tile_msg(t):
            feat = feat_by_t.pop(t)
            msg = msgp.tile([T, 144], bf16, tag="msg", name=f"msg{t}")
            featc = msgp.tile([T, NLV], bf16, tag="featc", name=f"featc{t}")
            nc.vector.tensor_copy(featc[:], feat)
            nc.gpsimd.tensor_copy(msg[:, 0:16], featc[:, 0:16])
            nc.gpsimd.tensor_tensor(
                msg[:, 16:64].rearrange("p (v m) -> p v m", m=3),
                featc[:, 16:32]
                .rearrange("p (v o) -> p v o", o=1)
                .to_broadcast([T, 16, 3]),
                sh_all[:, t, 1:4]
                .rearrange("p (o m) -> p o m", o=1)
                .to_broadcast([T, 16, 3]),
                op=OP.mult,
            )
            nc.gpsimd.tensor_tensor(
                msg[:, 64:144].rearrange("p (v m) -> p v m", m=5),
                featc[:, 32:48]
                .rearrange("p (v o) -> p v o", o=1)
                .to_broadcast([T, 16, 5]),
                sh_all[:, t, 4:9]
                .rearrange("p (o m) -> p o m", o=1)
                .to_broadcast([T, 16, 5]),
                op=OP.mult,
            )
            msg_by_t[t] = msg

        def emit_tile_scatter(t):
            msg = msg_by_t.pop(t)
            nc.sync.dma_start(out_d.ap()[:, t, :], msg[:])

        emit_mm1(0)
        for t in range(NT):
            # tail work for old tiles first: all inputs are >=1 iteration old,
            # so no engine stalls at the head of its in-order stream
            if t >= 3:
                emit_tile_reduce(t - 3)
                emit_tile_msg(t - 3)
            emit_tile_head(t)
            if (t + 2) % 4 == 0 and t + 2 < NT:
                emit_mm1((t + 2) // 4)
            if t >= 4:
                emit_tile_scatter(t - 4)
            if t == 1:
                emit_sh_part2()
        for t in range(NT - 3, NT):
            emit_tile_reduce(t)
            emit_tile_msg(t)
        for t in range(NT - 4, NT):
            emit_tile_scatter(t)

        # output: chunked DMAs, small final chunk so the tail drains fast
        pass

    nc.compile()
    return nc


def _get_program():
    global _PROG
    if _PROG is None:
        _PROG = _build_program()
    return _PROG


def _prep_core(c, h_src, edge_vec, edge_scalars, src_idx, dst_idx, inv_cnt):
    """Shard + sort + gather + one-hot build for one core (partition-major)."""
    import ml_dtypes

    bf = ml_dtypes.bfloat16
    lo, hi = c * EC, (c + 1) * EC
    d = dst_idx[lo:hi]
    order = np.argsort(d, kind="stable")
    d_s = d[order]
    s_s = src_idx[lo:hi][order]

    esT = np.zeros((ESD, EP), np.float32)
    esT[:, :EC] = edge_scalars[lo:hi][order].T
    # x rows pre-scaled by 1/cnt[dst] (folds the scatter-mean divide); pads 0
    x = np.zeros((EP, MUL_SRC), np.float32)
    x[:EC] = h_src[s_s] * inv_cnt[d_s][:, None]
    xrep = np.concatenate([x.T, x.T], axis=0)  # [128, EP]
    ev = np.zeros((EP, 3), np.float32)
    ev[:EC] = edge_vec[lo:hi][order]
    ev[EC:, 0] = 1.0

    labels = np.full(EP, N_DST, np.int64)
    labels[:EC] = d_s

    # partition-major device layouts: [p, t, ...] = row t*T + p
    def pmaj(a):
        return np.ascontiguousarray(a.reshape(NT, T, -1).transpose(1, 0, 2))

    return (
        {
            "esT": esT.astype(bf),
            "xrep": np.ascontiguousarray(xrep).astype(bf),
            "ev": pmaj(ev),
        },
        labels,
    )


def kernel(**inputs):
    import ml_dtypes

    from concourse import bass_utils

    bf = ml_dtypes.bfloat16

    h_src = np.asarray(inputs["h_src"], np.float32)
    edge_vec = np.asarray(inputs["edge_vec"], np.float32)
    edge_scalars = np.asarray(inputs["edge_scalars"], np.float32)
    W1 = np.asarray(inputs["W1"], np.float32)
    b1 = np.asarray(inputs["b1"], np.float32)
    W2 = np.asarray(inputs["W2"], np.float32)
    b2 = np.asarray(inputs["b2"], np.float32)
    src_idx = np.asarray(inputs["src_idx"]).astype(np.int64)
    dst_idx = np.asarray(inputs["dst_idx"]).astype(np.int64)
    n_dst = int(inputs["n_dst"])
    assert n_dst == N_DST

    nc = _get_program()

    cnt = np.bincount(dst_idx, minlength=N_DST)
    inv_cnt = (1.0 / np.maximum(cnt, 1)).astype(np.float32)

    # weights in (l,v,u) column order, pre-scaled by 1/sqrt(64)
    scale = 1.0 / np.sqrt(MUL_SRC)
    W2p = (
        W2.reshape(HID, N_PATHS, MUL_SRC, MUL_DST).transpose(0, 1, 3, 2) * scale
    ).reshape(HID, WCOLS)
    # bias path: feat += x @ b2r with b2r = b2 permuted to (u, l, v).  The
    # device does it as matmul(lhsT=xrep, rhs=b2r-padded); xrep rows 64:128
    # duplicate rows 0:64, so the rhs pads rows 64:128 with zeros.
    b2r = np.zeros((HID, NLV), np.float32)
    b2r[0:MUL_SRC] = (
        b2.reshape(N_PATHS, MUL_SRC, MUL_DST).transpose(1, 0, 2) * scale
    ).reshape(MUL_SRC, NLV)

    # E2: rows 0:64 -> col 0, rows 64:128 -> col 1
    e2 = np.zeros((HID, 2), np.float32)
    e2[0:MUL_SRC, 0] = 1.0
    e2[MUL_SRC:, 1] = 1.0

    shc = np.broadcast_to(
        np.array(
            [SQ3, SQ3, SQ3, SQ15, SQ15, 0.5 * SQ5, SQ15, 0.5 * SQ15], np.float32
        ),
        (T, 8),
    ).copy()

    shared = {
        "W1": np.ascontiguousarray(W1).astype(bf),
        "b1": b1.reshape(HID, 1).astype(np.float32),
        "W2p": W2p.astype(bf),
        "b2r": b2r.astype(bf),
        "e2": e2.astype(bf),
        "shc": shc,
    }

    in_maps = []
    labels_all = []
    for c in range(N_CORES):
        m, labels = _prep_core(
            c, h_src, edge_vec, edge_scalars, src_idx, dst_idx, inv_cnt
        )
        m.update(shared)
        in_maps.append(m)
        labels_all.append(labels)

    import time

    t0 = time.perf_counter()
    res = bass_utils.run_bass_kernel_spmd(nc, in_maps, core_ids=list(range(N_CORES)))
    t1 = time.perf_counter()
    kernel.last_device_wall_s = t1 - t0

    # outp is [T, NT, 144] partition-major; row (t, p) lives at [p, t, :]
    rows = np.concatenate(
        [
            res.results[c]["outp"]
            .astype(np.float32)
            .transpose(1, 0, 2)
            .reshape(NT * T, 144)
            for c in range(N_CORES)
        ],
        axis=0,
    )
    labels = np.concatenate(labels_all)

    order = np.argsort(labels, kind="stable")
    lab_s = labels[order]
    rows_s = rows[order]
    starts = np.concatenate(([0], np.flatnonzero(np.diff(lab_s)) + 1))
    sums = np.add.reduceat(rows_s, starts, axis=0)
    out = np.zeros((N_DST + 1, 144), np.float32)
    out[lab_s[starts]] = sums
    return out[:N_DST]


# revision 7
# speedup vs baseline: 1.1012x; 1.0065x over previous
"""Trainium2 Bass kernel for EquivariantTPConv (gnn_message_passing), v6.

Computation per edge e:
  sh  = SH_l012(edge_vec[e])                                  # [9]
  w   = (silu(edge_scalars[e] @ W1 + b1) @ W2 + b2)           # [3*64*16]
  x   = h_src[src_idx[e]]                                     # [64]
  feat[l,v] = sum_u x[u] * w[l,u,v] / 8                       # [3,16]
  msg = concat_l (feat[:,l,:,None] * sh_l[None,:])            # [144]
  out[d] = mean over {e: dst_idx[e]==d} msg[e]                # [n_dst,144]

v2 strategy (vs the v1 elementwise pipeline):
  - mm2 is emitted TRANSPOSED: 24 matmuls per tile with lhsT = W2p column
    block [128h, 128] and rhs = h2 [128h, 128e], giving wT[(l,v,u)-slice,
    e] in PSUM.  Each 128-partition block holds 2 (l,v) pairs x 64 u.
  - PSUM->SBUF bf16 cast of wT is split between ScalarE and GpSimd.
  - VectorE multiplies by xrep (x^T duplicated on 128 partitions, host-
    prepared, pre-scaled by 1/cnt[dst]) in 2x bf16 mode -> prodT.
  - The u-reduction is 24 tiny PE matmuls: lhsT = prodT block [128, 128e],
    rhs = E2 [128, 2] block-ones -> feat[T, 2] each, written into a shared
    feat PSUM region.  The b2 bias lands there too via one extra matmul
    with lhsT = xrep and rhs = b2 rows (b2 pre-divided into the two
    64-row halves).
  - msg outer products with SH on GpSimd read feat straight from PSUM;
    scatter uses the one-hot matmul; 1/cnt is pre-folded into xrep so the
    eviction is a plain copy on VectorE.
  - Host unshard: segment-sum of slot rows across tiles/cores (unchanged).
"""

import sys

for _p in ("/opt/trn_rl_repo", "/root/.axon_site/_ro/trn_rl_repo"):
    if _p not in sys.path:
        sys.path.append(_p)

import numpy as np

MUL_SRC = 64
MUL_DST = 16
N_PATHS = 3
SQ3 = 3.0 ** 0.5
SQ5 = 5.0 ** 0.5
SQ15 = 15.0 ** 0.5

N_CORES = 8
E_TOT = 50000
N_SRC = 10000
N_DST = 10000
ESD = 32
HID = 128
WCOLS = N_PATHS * MUL_DST * MUL_SRC  # 3072 (perm layout (l,v,u))
NLV = N_PATHS * MUL_DST  # 48

T = 128  # edges per tile
BLK = 512  # edges per full MM1 block (last block is a 128-edge tail)
EC = E_TOT // N_CORES  # 6250 edges per core
NT = (EC + T - 1) // T  # 49 tiles
EP = NT * T  # 6272 padded edges per core
NB = (EP + BLK - 1) // BLK  # 13 blocks, last one covers a single tile

NSB = WCOLS // T  # 24 (l,v,u) column blocks of 128 per tile
NSH = NSB // 2  # 12 blocks per half
HW = NSH * T  # 1536 cols per half

# per half: blocks 0..7 (1024 cols) go ScalarE-cast + VectorE-mult via pwS;
# blocks 8..11 (512 cols) go GpSimd fused cast*mult straight from pwP PSUM.
NS_S = 8  # pwS blocks per half
CS = NS_S * T  # 1024
NS_P = NSH - NS_S  # 4
CP = NS_P * T  # 512

_PROG = None  # cached compiled program


def _build_program():
    from contextlib import ExitStack

    import concourse.tile as tile
    from concourse import bacc, mybir

    f32 = mybir.dt.float32
    bf16 = mybir.dt.bfloat16
    AF = mybir.ActivationFunctionType
    OP = mybir.AluOpType
    AX = mybir.AxisListType

    nc = bacc.Bacc(
        "TRN2",
        target_bir_lowering=False,
        debug=False,
        enable_asserts=False,
        num_devices=N_CORES,
    )

    # DRAM inputs; all big per-core tensors are partition-major on the host.
    esT_d = nc.dram_tensor("esT", [ESD, EP], bf16, kind="ExternalInput")
    xrep_d = nc.dram_tensor("xrep", [HID, EP], bf16, kind="ExternalInput")
    ev_d = nc.dram_tensor("ev", [T, NT, 3], f32, kind="ExternalInput")
    W1_d = nc.dram_tensor("W1", [ESD, HID], bf16, kind="ExternalInput")
    b1_d = nc.dram_tensor("b1", [HID, 1], f32, kind="ExternalInput")
    W2_d = nc.dram_tensor("W2p", [HID, WCOLS], bf16, kind="ExternalInput")
    b2_d = nc.dram_tensor("b2r", [HID, NLV], bf16, kind="ExternalInput")
    e2_d = nc.dram_tensor("e2", [HID, 2], bf16, kind="ExternalInput")
    shc_d = nc.dram_tensor("shc", [T, 8], f32, kind="ExternalInput")
    out_d = nc.dram_tensor("outp", [T, NT, 144], bf16, kind="ExternalOutput")

    with ExitStack() as ctx:
        tc = ctx.enter_context(tile.TileContext(nc))

        const = ctx.enter_context(tc.tile_pool(name="const", bufs=1))
        shp = ctx.enter_context(tc.tile_pool(name="shp", bufs=1))
        h2pool = ctx.enter_context(tc.tile_pool(name="h2pool", bufs=2))
        wbp = ctx.enter_context(tc.tile_pool(name="wbp", bufs=3))
        prp = ctx.enter_context(tc.tile_pool(name="prp", bufs=3))
        msgp = ctx.enter_context(tc.tile_pool(name="msgp", bufs=10))
        ph1 = ctx.enter_context(tc.tile_pool(name="ph1", bufs=1, space="PSUM"))
        pwt = ctx.enter_context(tc.tile_pool(name="pwt", bufs=2, space="PSUM"))
        pfs = ctx.enter_context(tc.tile_pool(name="pfs", bufs=1, space="PSUM"))

        # ---- resident loads; SP carries the edge streams, GpSimd's queue
        # carries the weights, ordered by first use ----
        W1s = const.tile([ESD, HID], bf16)
        nc.gpsimd.dma_start(W1s[:], W1_d.ap())
        b1s = const.tile([HID, 1], f32)
        nc.gpsimd.dma_start(b1s[:], b1_d.ap())
        W2s = const.tile([HID, WCOLS], bf16)
        nc.gpsimd.dma_start(W2s[:, 0:1024], W2_d.ap()[:, 0:1024])
        nc.gpsimd.dma_start(W2s[:, 1024:], W2_d.ap()[:, 1024:])
        b2s = const.tile([HID, NLV], bf16)
        nc.gpsimd.dma_start(b2s[:], b2_d.ap())
        e2s = const.tile([HID, 2], bf16)
        nc.gpsimd.dma_start(e2s[:], e2_d.ap())
        shcs = const.tile([T, 8], f32)
        nc.gpsimd.dma_start(shcs[:], shc_d.ap())

        es_all = const.tile([ESD, EP], bf16)
        nc.sync.dma_start(es_all[:, 0:BLK], esT_d.ap()[:, 0:BLK])
        xrep_all = const.tile([HID, EP], bf16)
        nc.sync.dma_start(xrep_all[:, 0 : 4 * T], xrep_d.ap()[:, 0 : 4 * T])
        ev_all = const.tile([T, NT, 3], f32)
        nc.gpsimd.dma_start(ev_all[:], ev_d.ap())

        # interleaved chunked loads: keep both streams ~4 tiles ahead of
        # the pipeline instead of one monolithic tail DMA each
        chunk = 8 * T
        lo = 4 * T
        while lo < EP:
            hi = min(lo + chunk, EP)
            nc.sync.dma_start(es_all[:, lo:hi], esT_d.ap()[:, lo:hi])
            nc.sync.dma_start(xrep_all[:, lo:hi], xrep_d.ap()[:, lo:hi])
            lo = hi
        negone = const.tile([T, 1], f32)
        nc.vector.memset(negone[:], -1.0)


        # ---- SH prologue: all edges at once, [128, NT, k] layouts ----
        sq_all = shp.tile([T, NT, 3], f32)
        nc.vector.tensor_tensor(sq_all[:], ev_all[:], ev_all[:], op=OP.mult)
        r2_all = shp.tile([T, NT], f32)
        nc.vector.tensor_reduce(r2_all[:], sq_all[:], axis=AX.X, op=OP.add)
        rn_all = shp.tile([T, NT], f32)
        nc.scalar.activation(rn_all[:], r2_all[:], AF.Sqrt)

        def bc(ap_, shape):
            return ap_.to_broadcast(shape)

        sh_all = shp.tile([T, NT, 9], f32)

        def emit_sh_part2():
            inv_all = shp.tile([T, NT], f32)
            nc.vector.reciprocal(inv_all[:], rn_all[:])
            inv2_all = shp.tile([T, NT], f32)
            nc.vector.tensor_tensor(inv2_all[:], inv_all[:], inv_all[:], op=OP.mult)
            i1 = inv_all[:].rearrange("p (t o) -> p t o", o=1)
            i2 = inv2_all[:].rearrange("p (t o) -> p t o", o=1)
            nc.vector.tensor_tensor(
                sh_all[:, :, 1:4], ev_all[:], bc(i1, [T, NT, 3]), op=OP.mult
            )
            pq_all = shp.tile([T, NT, 2], f32)
            nc.vector.tensor_tensor(
                pq_all[:], ev_all[:, :, 0:2], ev_all[:, :, 1:3], op=OP.mult
            )
            nc.vector.tensor_tensor(
                sh_all[:, :, 4:6], pq_all[:], bc(i2, [T, NT, 2]), op=OP.mult
            )
            t6_all = shp.tile([T, NT], f32)
            nc.vector.tensor_tensor(
                t6_all[:].rearrange("p (t o) -> p t o", o=1),
                sq_all[:, :, 2:3],
                i2,
                op=OP.mult,
            )
            nc.scalar.activation(
                sh_all[:, :, 6], t6_all[:], AF.Identity, bias=negone[:, 0:1], scale=3.0
            )
            xz_all = shp.tile([T, NT, 1], f32)
            nc.vector.tensor_tensor(
                xz_all[:], ev_all[:, :, 0:1], ev_all[:, :, 2:3], op=OP.mult
            )
            nc.vector.tensor_tensor(sh_all[:, :, 7:8], xz_all[:], i2, op=OP.mult)
            d2_all = shp.tile([T, NT, 1], f32)
            nc.vector.tensor_tensor(
                d2_all[:], sq_all[:, :, 0:1], sq_all[:, :, 1:2], op=OP.subtract
            )
            nc.vector.tensor_tensor(sh_all[:, :, 8:9], d2_all[:], i2, op=OP.mult)
            shc3 = shcs[:].rearrange("p (o c) -> p o c", o=1)
            nc.vector.tensor_tensor(
                sh_all[:, :, 1:9], sh_all[:, :, 1:9], bc(shc3, [T, NT, 8]), op=OP.mult
            )

        # ---- main pipeline ----
        # stage A(t): mm2 halves -> pwt PSUM; casts (S/P); mult (V) -> prodT
        # stage B(t-1): bias-mm + 24 reduce-mms -> feat region of fp
        #               gpsimd msg outers from feat PSUM; l0 copy
        # stage C(t-2): scatter-mm -> ps region of fp; V evict -> ob_all
        prodT_by_t = {}
        feat_by_t = {}
        msg_by_t = {}

        # single PSUM bank shared by feat [0:48] and the scatter out [64:208]
        fp = pfs.tile([T, 512], f32, tag="fp", name="fp")

        def emit_mm1(b):
            nbt = min(4, NT - b * 4)
            bw = nbt * T
            h1 = ph1.tile([HID, BLK], f32, tag="h1", name=f"h1_{b}")
            nc.tensor.matmul(
                h1[:, 0:bw],
                W1s[:],
                es_all[:, b * BLK : b * BLK + bw],
                start=True,
                stop=True,
            )
            h2 = h2pool.tile([HID, BLK], bf16, tag="h2", name=f"h2_{b}")
            nc.scalar.activation(h2[:, 0:bw], h1[:, 0:bw], AF.Silu, bias=b1s[:, 0:1])
            emit_mm1.h2_by_b[b] = h2

        emit_mm1.h2_by_b = {}

        def emit_tile_head(t):
            b, q = divmod(t, 4)
            h2 = emit_mm1.h2_by_b[b]
            if q == 3 and b - 1 in emit_mm1.h2_by_b:
                del emit_mm1.h2_by_b[b - 1]
            prodT = prp.tile([HID, WCOLS], bf16, tag="prodT", name=f"prodT{t}")
            xe = xrep_all[:, t * T : (t + 1) * T]
            xb1 = xe.rearrange("p (o e) -> p o e", o=1)
            rhs = h2[:, q * T : (q + 1) * T]
            # all pwS matmuls first so the S casts never wait behind pwP work
            pwS_h, pwP_h = [], []
            for hh in range(2):
                pwS = pwt.tile([HID, CS], f32, tag="pwS", name=f"pwS{t}_{hh}", bufs=2)
                pwS_h.append(pwS)
                for j in range(NS_S):
                    s = hh * NSH + j
                    nc.tensor.matmul(
                        pwS[:, j * T : (j + 1) * T],
                        W2s[:, s * T : (s + 1) * T],
                        rhs,
                        start=True,
                        stop=True,
                    )
            for hh in range(2):
                pwP = pwt.tile([HID, CP], f32, tag="pwP", name=f"pwP{t}_{hh}", bufs=2)
                pwP_h.append(pwP)
                for j in range(NS_P):
                    s = hh * NSH + NS_S + j
                    nc.tensor.matmul(
                        pwP[:, j * T : (j + 1) * T],
                        W2s[:, s * T : (s + 1) * T],
                        rhs,
                        start=True,
                        stop=True,
                    )
            for hh in range(2):
                wb = wbp.tile([HID, CS], bf16, tag="wb", name=f"wb{t}_{hh}")
                nc.scalar.activation(wb[:], pwS_h[hh][:], AF.Copy)
                # GpSimd fused cast*mult straight from PSUM for blocks 8..11
                pp = prodT[:, hh * HW + CS : (hh + 1) * HW].rearrange(
                    "p (s e) -> p s e", e=T
                )
                nc.vector.tensor_tensor(
                    pp,
                    pwP_h[hh][:].rearrange("p (s e) -> p s e", e=T),
                    xb1.to_broadcast([HID, NS_P, T]),
                    op=OP.mult,
                )
                # VectorE 2x bf16 multiply for the ScalarE-cast blocks
                po = prodT[:, hh * HW : hh * HW + CS - 384].rearrange(
                    "p (s e) -> p s e", e=T
                )
                nc.vector.tensor_tensor(
                    po,
                    wb[:, 0 : CS - 384].rearrange("p (s e) -> p s e", e=T),
                    xb1.to_broadcast([HID, NS_S - 3, T]),
                    op=OP.mult,
                )
                nc.gpsimd.tensor_tensor(
                    prodT[:, hh * HW + CS - 384 : hh * HW + CS].rearrange(
                        "p (s e) -> p s e", e=T
                    ),
                    wb[:, CS - 384 : CS].rearrange("p (s e) -> p s e", e=T),
                    xb1.to_broadcast([HID, 3, T]),
                    op=OP.mult,
                )
            prodT_by_t[t] = prodT

        def emit_tile_reduce(t):
            prodT = prodT_by_t.pop(t)
            feat = fp[:, 256 * (t % 2) : 256 * (t % 2) + NLV]
            # bias: lhsT = xrep (full 128 partitions), rhs = b2 rows (/1 in
            # rows 0:64, zeros in 64:128)
            nc.tensor.matmul(
                feat, xrep_all[:, t * T : (t + 1) * T], b2s[:], start=True, stop=False
            )
            off = 256 * (t % 2)
            for s in range(NSB):
                nc.tensor.matmul(
                    fp[:, off + 2 * s : off + 2 * s + 2],
                    prodT[:, s * T : (s + 1) * T],
                    e2s[:],
                    start=False,
                    stop=True,
                )
            feat_by_t[t] = feat

        def emit_tile_msg(t):
            feat = feat_by_t.pop(t)
            msg = msgp.tile([T, 144], bf16, tag="msg", name=f"msg{t}")
            featc = msgp.tile([T, NLV], bf16, tag="featc", name=f"featc{t}")
            nc.vector.tensor_copy(featc[:], feat)
            nc.gpsimd.tensor_copy(msg[:, 0:16], featc[:, 0:16])
            nc.gpsimd.tensor_tensor(
                msg[:, 16:64].rearrange("p (v m) -> p v m", m=3),
                featc[:, 16:32]
                .rearrange("p (v o) -> p v o", o=1)
                .to_broadcast([T, 16, 3]),
                sh_all[:, t, 1:4]
                .rearrange("p (o m) -> p o m", o=1)
                .to_broadcast([T, 16, 3]),
                op=OP.mult,
            )
            nc.gpsimd.tensor_tensor(
                msg[:, 64:144].rearrange("p (v m) -> p v m", m=5),
                featc[:, 32:48]
                .rearrange("p (v o) -> p v o", o=1)
                .to_broadcast([T, 16, 5]),
                sh_all[:, t, 4:9]
                .rearrange("p (o m) -> p o m", o=1)
                .to_broadcast([T, 16, 5]),
                op=OP.mult,
            )
            msg_by_t[t] = msg

        def emit_tile_scatter(t):
            msg = msg_by_t.pop(t)
            nc.sync.dma_start(out_d.ap()[:, t, :], msg[:])

        emit_mm1(0)
        for t in range(NT):
            # tail work for old tiles first: all inputs are >=1 iteration old,
            # so no engine stalls at the head of its in-order stream
            if t >= 3:
                emit_tile_reduce(t - 3)
                emit_tile_msg(t - 3)
            emit_tile_head(t)
            if (t + 2) % 4 == 0 and t + 2 < NT:
                emit_mm1((t + 2) // 4)
            if t >= 4:
                emit_tile_scatter(t - 4)
            if t == 1:
                emit_sh_part2()
        for t in range(NT - 3, NT):
            emit_tile_reduce(t)
            emit_tile_msg(t)
        for t in range(NT - 4, NT):
            emit_tile_scatter(t)

        # output: chunked DMAs, small final chunk so the tail drains fast
        pass

    nc.compile()
    return nc


def _get_program():
    global _PROG
    if _PROG is None:
        _PROG = _build_program()
    return _PROG


def _prep_core(c, h_src, edge_vec, edge_scalars, src_idx, dst_idx, inv_cnt):
    """Shard + sort + gather + one-hot build for one core (partition-major)."""
    import ml_dtypes

    bf = ml_dtypes.bfloat16
    lo, hi = c * EC, (c + 1) * EC
    d = dst_idx[lo:hi]
    order = np.argsort(d, kind="stable")
    d_s = d[order]
    s_s = src_idx[lo:hi][order]

    esT = np.zeros((ESD, EP), np.float32)
    esT[:, :EC] = edge_scalars[lo:hi][order].T
    # x rows pre-scaled by 1/cnt[dst] (folds the scatter-mean divide); pads 0
    x = np.zeros((EP, MUL_SRC), np.float32)
    x[:EC] = h_src[s_s] * inv_cnt[d_s][:, None]
    xrep = np.concatenate([x.T, x.T], axis=0)  # [128, EP]
    ev = np.zeros((EP, 3), np.float32)
    ev[:EC] = edge_vec[lo:hi][order]
    ev[EC:, 0] = 1.0

    labels = np.full(EP, N_DST, np.int64)
    labels[:EC] = d_s

    # partition-major device layouts: [p, t, ...] = row t*T + p
    def pmaj(a):
        return np.ascontiguousarray(a.reshape(NT, T, -1).transpose(1, 0, 2))

    return (
        {
            "esT": esT.astype(bf),
            "xrep": np.ascontiguousarray(xrep).astype(bf),
            "ev": pmaj(ev),
        },
        labels,
    )


def kernel(**inputs):
    import ml_dtypes

    from concourse import bass_utils

    bf = ml_dtypes.bfloat16

    h_src = np.asarray(inputs["h_src"], np.float32)
    edge_vec = np.asarray(inputs["edge_vec"], np.float32)
    edge_scalars = np.asarray(inputs["edge_scalars"], np.float32)
    W1 = np.asarray(inputs["W1"], np.float32)
    b1 = np.asarray(inputs["b1"], np.float32)
    W2 = np.asarray(inputs["W2"], np.float32)
    b2 = np.asarray(inputs["b2"], np.float32)
    src_idx = np.asarray(inputs["src_idx"]).astype(np.int64)
    dst_idx = np.asarray(inputs["dst_idx"]).astype(np.int64)
    n_dst = int(inputs["n_dst"])
    assert n_dst == N_DST

    nc = _get_program()

    cnt = np.bincount(dst_idx, minlength=N_DST)
    inv_cnt = (1.0 / np.maximum(cnt, 1)).astype(np.float32)

    # weights in (l,v,u) column order, pre-scaled by 1/sqrt(64)
    scale = 1.0 / np.sqrt(MUL_SRC)
    W2p = (
        W2.reshape(HID, N_PATHS, MUL_SRC, MUL_DST).transpose(0, 1, 3, 2) * scale
    ).reshape(HID, WCOLS)
    # bias path: feat += x @ b2r with b2r = b2 permuted to (u, l, v).  The
    # device does it as matmul(lhsT=xrep, rhs=b2r-padded); xrep rows 64:128
    # duplicate rows 0:64, so the rhs pads rows 64:128 with zeros.
    b2r = np.zeros((HID, NLV), np.float32)
    b2r[0:MUL_SRC] = (
        b2.reshape(N_PATHS, MUL_SRC, MUL_DST).transpose(1, 0, 2) * scale
    ).reshape(MUL_SRC, NLV)

    # E2: rows 0:64 -> col 0, rows 64:128 -> col 1
    e2 = np.zeros((HID, 2), np.float32)
    e2[0:MUL_SRC, 0] = 1.0
    e2[MUL_SRC:, 1] = 1.0

    shc = np.broadcast_to(
        np.array(
            [SQ3, SQ3, SQ3, SQ15, SQ15, 0.5 * SQ5, SQ15, 0.5 * SQ15], np.float32
        ),
        (T, 8),
    ).copy()

    shared = {
        "W1": np.ascontiguousarray(W1).astype(bf),
        "b1": b1.reshape(HID, 1).astype(np.float32),
        "W2p": W2p.astype(bf),
        "b2r": b2r.astype(bf),
        "e2": e2.astype(bf),
        "shc": shc,
    }

    in_maps = []
    labels_all = []
    for c in range(N_CORES):
        m, labels = _prep_core(
            c, h_src, edge_vec, edge_scalars, src_idx, dst_idx, inv_cnt
        )
        m.update(shared)
        in_maps.append(m)
        labels_all.append(labels)

    import time

    t0 = time.perf_counter()
    res = bass_utils.run_bass_kernel_spmd(nc, in_maps, core_ids=list(range(N_CORES)))
    t1 = time.perf_counter()
    kernel.last_device_wall_s = t1 - t0

    # outp is [T, NT, 144] partition-major; row (t, p) lives at [p, t, :]
    rows = np.concatenate(
        [
            res.results[c]["outp"]
            .astype(np.float32)
            .transpose(1, 0, 2)
            .reshape(NT * T, 144)
            for c in range(N_CORES)
        ],
        axis=0,
    )
    labels = np.concatenate(labels_all)

    order = np.argsort(labels, kind="stable")
    lab_s = labels[order]
    rows_s = rows[order]
    starts = np.concatenate(([0], np.flatnonzero(np.diff(lab_s)) + 1))
    sums = np.add.reduceat(rows_s, starts, axis=0)
    out = np.zeros((N_DST + 1, 144), np.float32)
    out[lab_s[starts]] = sums
    return out[:N_DST]
